# revision 16
# baseline (speedup 1.0000x reference)
"""GIN decoder (segment_sum aggregation + 2-layer linear MLP) on 8 trn2 cores.

Key optimizations vs the v1 kernel:
  * The decoder MLP has NO nonlinearity, so the two Linears fold into one:
    out = h @ (W2 @ W1).T + (b1 @ W2.T + b2).  ~9x fewer matmul FLOPs.
    Wf^T is computed on-device (vocab-sharded across the 8 cores, ~340
    matmuls/core) and AllGathered (DRAM->DRAM) while aggregation runs.
  * Gathers use InstDMAGatherAnt (nc.gpsimd.dma_gather): one instruction per
    2304 edges instead of one indirect DMA per 128 edges.  The old path paid
    ~1us SWDGE fixed cost per 128-row gather (795us of serialized gpsimd
    time); dma_gather generates descriptors via the Q7 counter machine
    (~0.34ns/descriptor) and spreads across 4 SWDGE queues.
  * One-hot scatter masks are built with a single batched is_equal per
    128-node block (bf16, 2x DVE rate) instead of 36 small ops.

Per-core schedule (SPMD, data-parallel over dst nodes):
  1. Wf^T slice: core c computes Wf^T[:, c*1024:(c+1)*1024] = sum_k
     W1[k,:]^T W2^T[k, vslice] on the PE, with an extra lhsT tile holding b1
     broadcast 128-wide so psum tile 4 evicts as (W2@b1 + b2) replicated
     across partitions.  Slices are AllGathered to every core.
  2. Aggregation: edges host-bucketed by (core, 128-wide dst block), padded
     to 36*128 per bucket with zero-row edges; each half-block dma_gathers
     2304 x-rows by src id and scatter-adds into its dst block via one-hot
     matmuls in PSUM; block results are PE-transposed to feature-major hT.
  3. Fused output matmul: out[128 nodes, 8192] = hT.T @ Wf^T + bias, K=512,
     bias added at PSUM evict, output written bf16 node-major.  Interleaved
     with aggregation so the PE stays busy while the AllGather completes.

All matmuls bf16 with f32 PSUM accumulation; output bf16 -> host f32.
"""

import numpy as np
import ml_dtypes

P = 128
N_NODES = 20000
HIDDEN = 512
MIDDLE = 4352
VOCAB = 8192
NCORES = 8
ND = N_NODES // NCORES          # 2500 nodes per core
NDP = 2560                      # padded to 20*128
KSRC = 20096                    # 157*128, gather source rows padded
BF16 = ml_dtypes.bfloat16

NBLK = NDP // P                 # 20 dst blocks of 128 per core
T_TILES = 36                    # 128-edge tiles per dst block (4608 cap)
ZERO_ROW = N_NODES              # gather target row holding zeros

VSL = VOCAB // NCORES           # 1024 vocab cols of Wf computed per core
KT = MIDDLE // P                # 34 contraction tiles for the Wf fold
FT = HIDDEN // P                # 4 feature tiles
MT_W = FT + 1                   # 4 feat tiles + 1 bias tile in Wf^T slice
NVT = VOCAB // 512              # 16 vocab tiles in fused matmul

CC_AFTER_BLOCK = 6              # emit AllGather after this many agg blocks
DELAY = 8                       # agg blocks in flight before first fused mm
NSWQ = 4                        # SWDGE queues for gather round-robin
USE_DMA_GATHER = True           # bisect flag: False -> per-tile indirect DMAs

_BUILT = {}
LAST_RESULTS = None             # state of the last run (for test.py)


def _pack(a):
    """[K, M] row-major -> partition-tiled [P, K//P, M] (row r -> [r%P, r//P, :])."""
    K, M = a.shape
    assert K % P == 0, (K, M)
    return np.ascontiguousarray(a.reshape(K // P, P, M).transpose(1, 0, 2))


def _unpack(a):
    """[P, MB, N] -> [MB*P, N]."""
    Pp, MB, N = a.shape
    return np.ascontiguousarray(a.transpose(1, 0, 2)).reshape(MB * Pp, N)


def _build(T=T_TILES):
    key = ("nc", T)
    if key in _BUILT:
        return _BUILT[key]
    from contextlib import ExitStack
    from concourse import bacc, mybir
    import concourse.bass as bass
    import concourse.tile as tile
    from concourse.masks import make_identity

    TH = T // 2                 # tiles per half-block gather
    NIDX = TH * P               # idxs per dma_gather (2304)

    dt = mybir.dt
    nc = bacc.Bacc("TRN2", target_bir_lowering=False, debug=False,
                   num_devices=NCORES, num_swdge_queues=NSWQ)

    x_rows = nc.dram_tensor("x_rows", [KSRC, HIDDEN], dt.bfloat16,
                            kind="ExternalInput").ap()
    gidx = nc.dram_tensor("gidx", [NBLK * 2, P, NIDX // 16], dt.int16,
                          kind="ExternalInput").ap()
    src_ids = nc.dram_tensor("src_ids", [NBLK, P, T], dt.int32,
                             kind="ExternalInput").ap()
    dst_ids = nc.dram_tensor("dst_ids", [NBLK, P, T, 1], dt.bfloat16,
                             kind="ExternalInput").ap()
    colidx = nc.dram_tensor("colidx", [P, P], dt.bfloat16,
                            kind="ExternalInput").ap()
    w1b_kxm = nc.dram_tensor("w1b_kxm", [P, KT, HIDDEN + P], dt.bfloat16,
                             kind="ExternalInput").ap()
    w2t_kxn = nc.dram_tensor("w2t_kxn", [P, KT, VSL], dt.bfloat16,
                             kind="ExternalInput").ap()
    b2rep = nc.dram_tensor("b2rep", [P, VSL], dt.bfloat16,
                           kind="ExternalInput").ap()
    out_pvn = nc.dram_tensor("out_pvn", [P, NBLK, VOCAB], dt.bfloat16,
                             kind="ExternalOutput").ap()

    with tile.TileContext(nc) as tc:
        with ExitStack() as ctx:
            dram = ctx.enter_context(tc.tile_pool(name="dram", bufs=1,
                                                  space="DRAM"))
            wf_slice = dram.tile([P, MT_W, VSL], dt.bfloat16)
            wf_all = dram.tile([NCORES, P, MT_W, VSL], dt.bfloat16,
                               addr_space="Shared")

            const = ctx.enter_context(tc.tile_pool(name="const", bufs=1))
            idp = ctx.enter_context(tc.tile_pool(name="idx", bufs=1))

            # index/one-hot operand loads ride the scalar (ACT) HWDGE queue
            # so they don't delay the weight loads on the sync queue
            def load_ids(b):
                gi = []
                for hb in range(2):
                    g = idp.tile([P, NIDX // 16], dt.int16,
                                 name=f"gidx{2 * b + hb}")
                    nc.scalar.dma_start(g[:], gidx[2 * b + hb])
                    gi.append(g)
                did = idp.tile([P, T, 1], dt.bfloat16, name=f"did{b}")
                nc.scalar.dma_start(did[:], dst_ids[b])
                sid = None
                if not USE_DMA_GATHER:
                    sid = idp.tile([P, T], dt.int32, name=f"sid{b}")
                    nc.scalar.dma_start(sid[:], src_ids[b])
                return gi, did, sid

            ids = [load_ids(b) for b in range(NBLK)]

            colidx_sb = const.tile([P, 1, P], dt.bfloat16)
            nc.sync.dma_start(colidx_sb[:, 0, :], colidx[:])
            ident = const.tile([P, P], dt.float32)
            make_identity(nc, ident[:])

            # agg-phase pools allocated BEFORE the Wf weight pools so the
            # gather buffers don't alias the Wf weights' SBUF space (aliasing
            # would stall the first gathers until every Wf matmul retires).
            hp = ctx.enter_context(tc.tile_pool(name="hT", bufs=DELAY + 2))
            gp = ctx.enter_context(tc.tile_pool(
                name="gath", bufs=2 if USE_DMA_GATHER else 12))
            ohp = ctx.enter_context(tc.tile_pool(name="oneh", bufs=2))
            hsbp = ctx.enter_context(tc.tile_pool(name="hsb", bufs=2))
            outp = ctx.enter_context(tc.tile_pool(name="outp", bufs=4))

            # ---- Phase 1: Wf^T slice = W1(+b1) x W2^T[:, vslice] ----
            # Bias tile (mt=4) emitted first so its DVE bias-add doesn't
            # queue behind the whole Wf phase and stall the agg one-hots.
            with ExitStack() as pctx:
                wpool = pctx.enter_context(tc.tile_pool(name="wf_w", bufs=1))
                w1b_sb = wpool.tile([P, KT, HIDDEN + P], dt.bfloat16)
                nc.sync.dma_start(w1b_sb[:], w1b_kxm[:])
                w2t_sb = wpool.tile([P, KT, VSL], dt.bfloat16)
                nc.sync.dma_start(w2t_sb[:], w2t_kxn[:])
                b2_sb = wpool.tile([P, VSL], dt.bfloat16)
                nc.sync.dma_start(b2_sb[:], b2rep[:])
                wfps = pctx.enter_context(tc.tile_pool(name="wf_ps", bufs=2,
                                                       space="PSUM"))
                wev = pctx.enter_context(tc.tile_pool(name="wf_ev", bufs=3))
                for mt in [MT_W - 1] + list(range(MT_W - 1)):
                    for v in range(VSL // 512):
                        ps = wfps.tile([P, 512], dt.float32, space="PSUM")
                        for kt in range(KT):
                            nc.tensor.matmul(
                                ps[:],
                                lhsT=w1b_sb[:, kt, mt * P:(mt + 1) * P],
                                rhs=w2t_sb[:, kt, v * 512:(v + 1) * 512],
                                start=(kt == 0), stop=(kt == KT - 1))
                        ev = wev.tile([P, 512], dt.bfloat16)
                        if mt == MT_W - 1:
                            nc.vector.tensor_tensor(
                                out=ev[:], in0=ps[:],
                                in1=b2_sb[:, v * 512:(v + 1) * 512],
                                op=mybir.AluOpType.add)
                        else:
                            nc.scalar.copy(ev[:], ps[:])
                        nc.sync.dma_start(
                            wf_slice[:, mt, v * 512:(v + 1) * 512], ev[:])

            # ---- Phase 2+3: aggregation + fused matmul, pipelined ----
            # wf_sb reuses the SBUF space of the (now-freed) Wf weight pool;
            # its writes depend on the AllGather anyway.
            main = ctx.enter_context(tc.tile_pool(name="main", bufs=1))
            wf_sb = main.tile([P, MT_W, VOCAB], dt.bfloat16)
            aggps = ctx.enter_context(tc.tile_pool(name="agg_ps", bufs=2,
                                                   space="PSUM"))
            tpps = ctx.enter_context(tc.tile_pool(name="tp_ps", bufs=2,
                                                  space="PSUM"))
            fps = ctx.enter_context(tc.tile_pool(name="f_ps", bufs=3,
                                                 space="PSUM"))

            hts = []

            def agg_block(b):
                gi, did, sid = ids[b]
                oh = ohp.tile([P, T, P], dt.bfloat16, name="onehot")
                nc.vector.tensor_tensor(
                    out=oh[:], in0=did[:].to_broadcast([P, T, P]),
                    in1=colidx_sb[:].to_broadcast([P, T, P]),
                    op=mybir.AluOpType.is_equal)
                ps = aggps.tile([P, HIDDEN], dt.float32, space="PSUM")
                if USE_DMA_GATHER:
                    gs = []
                    for hb in range(2):
                        g = gp.tile([P, TH, HIDDEN], dt.bfloat16, name="gather")
                        nc.gpsimd.dma_gather(
                            g[:], x_rows[:], gi[hb][:], NIDX, NIDX, HIDDEN,
                            single_packet=False,
                            queue_num=(2 * b + hb) % NSWQ)
                        gs.append(g)
                    for t in range(T):
                        nc.tensor.matmul(ps[:], lhsT=oh[:, t, :],
                                         rhs=gs[t // TH][:, t % TH, :],
                                         start=(t == 0), stop=(t == T - 1))
                else:
                    for t in range(T):
                        g = gp.tile([P, HIDDEN], dt.bfloat16, name="gather")
                        nc.gpsimd.indirect_dma_start(
                            out=g[:], out_offset=None, in_=x_rows[:],
                            in_offset=bass.IndirectOffsetOnAxis(
                                ap=sid[:, t:t + 1], axis=0))
                        nc.tensor.matmul(ps[:], lhsT=oh[:, t, :],
                                         rhs=g[:],
                                         start=(t == 0), stop=(t == T - 1))
                hsb = hsbp.tile([P, HIDDEN], dt.float32, name="hsb")
                nc.scalar.copy(hsb[:], ps[:])
                ht = hp.tile([P, FT, P], dt.bfloat16, name="ht")
                for j in range(FT):
                    tp = tpps.tile([P, P], dt.float32, space="PSUM")
                    nc.tensor.transpose(out=tp[:], in_=hsb[:, j * P:(j + 1) * P],
                                        identity=ident[:])
                    nc.vector.tensor_copy(ht[:, j, :], tp[:])
                hts.append(ht)

            def fused_block(b):
                ht = hts[b]
                for v in range(NVT):
                    ps = fps.tile([P, 512], dt.float32, space="PSUM")
                    for k in range(FT):
                        nc.tensor.matmul(
                            ps[:], lhsT=ht[:, k, :],
                            rhs=wf_sb[:, k, v * 512:(v + 1) * 512],
                            start=(k == 0), stop=(k == FT - 1))
                    ob = outp.tile([P, 512], dt.bfloat16, name="ob")
                    nc.vector.tensor_tensor(
                        out=ob[:], in0=ps[:],
                        in1=wf_sb[:, MT_W - 1, v * 512:(v + 1) * 512],
                        op=mybir.AluOpType.add)
                    nc.sync.dma_start(out_pvn[:, b, v * 512:(v + 1) * 512],
                                      ob[:])

            def emit_collective():
                nc.gpsimd.collective_compute(
                    "AllGather", mybir.AluOpType.bypass,
                    replica_groups=[list(range(NCORES))],
                    ins=[wf_slice[:].opt()],
                    outs=[wf_all[:].opt()])

            def emit_wf_load():
                for c in range(NCORES):
                    nc.sync.dma_start(wf_sb[:, :, c * VSL:(c + 1) * VSL],
                                      wf_all[c])

            for b in range(NBLK):
                agg_block(b)
                if b == CC_AFTER_BLOCK:
                    emit_collective()
                if b == DELAY - 1:
                    emit_wf_load()
                if b >= DELAY:
                    fused_block(b - DELAY)
            for b in range(NBLK - DELAY, NBLK):
                fused_block(b)

    nc.compile()
    _BUILT[key] = nc
    return nc


def _make_runner(T=T_TILES):
    """Build (once) a cached sharded-jit callable over the 8 cores.

    Returns dict with: fn(ins_dev, outs_prev) -> outs, names, avals, mesh,
    sharding.  Outputs are donated back in as the next call's (fully
    overwritten) output buffers, so steady-state calls move no host data.
    """
    rkey = ("runner", T)
    if rkey in _BUILT:
        return _BUILT[rkey]
    import jax
    from jax.experimental.shard_map import shard_map
    from jax.sharding import Mesh, NamedSharding, PartitionSpec
    from concourse import bass2jax, mybir

    nc = _build(T)
    bass2jax.install_neuronx_cc_hook()

    pid_name = (nc.partition_id_tensor.name
                if nc.partition_id_tensor is not None else None)
    in_names, out_names, out_avals = [], [], []
    for alloc in nc.m.functions[0].allocations:
        if not isinstance(alloc, mybir.MemoryLocationSet):
            continue
        name = alloc.memorylocations[0].name
        if alloc.kind == "ExternalInput":
            if name != pid_name:
                in_names.append(name)
        elif alloc.kind == "ExternalOutput":
            out_names.append(name)
            out_avals.append(jax.core.ShapedArray(
                tuple(alloc.tensor_shape), mybir.dt.np(alloc.dtype)))
    n_params = len(in_names)
    all_names = in_names + out_names
    if pid_name is not None:
        all_names = all_names + [pid_name]
    donate = tuple(range(n_params, n_params + len(out_names)))

    def _body(*args):
        operands = list(args)
        if pid_name is not None:
            operands.append(bass2jax.partition_id_tensor())
        outs = bass2jax._bass_exec_p.bind(
            *operands,
            out_avals=tuple(out_avals),
            in_names=tuple(all_names),
            out_names=tuple(out_names),
            lowering_input_output_aliases=(),
            sim_require_finite=True,
            sim_require_nnan=True,
            nc=nc,
        )
        return tuple(outs)

    devices = jax.devices()[:NCORES]
    mesh = Mesh(np.asarray(devices), ("core",))
    spec = PartitionSpec("core")
    in_specs = (spec,) * (n_params + len(out_names))
    out_specs = (spec,) * len(out_names)
    fn = jax.jit(
        shard_map(_body, mesh=mesh, in_specs=in_specs, out_specs=out_specs,
                  check_rep=False),
        donate_argnums=donate, keep_unused=True,
    )
    sharding = NamedSharding(mesh, spec)
    runner = dict(fn=fn, in_names=in_names, out_names=out_names,
                  out_avals=out_avals, sharding=sharding, mesh=mesh)
    _BUILT[rkey] = runner
    return runner


def _prep_device_inputs(in_maps, T=T_TILES):
    """device_put the concatenated per-core inputs; returns (ins_dev, zeros)."""
    import jax
    r = _make_runner(T)
    concat = [np.concatenate([m[name] for m in in_maps], axis=0)
              for name in r["in_names"]]
    ins_dev = [jax.device_put(a, r["sharding"]) for a in concat]
    zeros = [
        jax.jit(lambda a=av: jax.numpy.zeros(
            (NCORES * a.shape[0], *a.shape[1:]), a.dtype),
            out_shardings=r["sharding"])()
        for av in r["out_avals"]
    ]
    jax.block_until_ready(ins_dev + zeros)
    return ins_dev, zeros


def _run_once(ins_dev, out_bufs, T=T_TILES):
    import jax
    r = _make_runner(T)
    outs = r["fn"](*ins_dev, *out_bufs)
    jax.block_until_ready(outs)
    return outs


def host_pack(x, edge_index, W1, b1, W2, b2):
    """Host-side packing: returns (in_maps, T)."""
    x = np.asarray(x, dtype=np.float32)
    edge_index = np.asarray(edge_index)
    W1 = np.asarray(W1, dtype=np.float32)
    b1 = np.asarray(b1, dtype=np.float32)
    W2 = np.asarray(W2, dtype=np.float32)
    b2 = np.asarray(b2, dtype=np.float32)

    src = edge_index[0].astype(np.int64)
    dst = edge_index[1].astype(np.int64)

    # --- host packing (index preprocessing + layout/dtype shuffles) ---
    x_rows = np.zeros((KSRC, HIDDEN), dtype=BF16)
    x_rows[:N_NODES] = x

    # Edge list incl. self-loops (the GIN (1+eps)*x_i term, eps=0), bucketed
    # by (core, dst-block of 128) and padded to T*128 per bucket with edges
    # from the all-zeros row.
    allsrc = np.concatenate([src, np.arange(N_NODES, dtype=np.int64)])
    alldst = np.concatenate([dst, np.arange(N_NODES, dtype=np.int64)])
    core = alldst // ND
    local = alldst % ND
    blk = local // P
    within = (local % P).astype(np.int32)
    bucket = core * NBLK + blk
    order = np.argsort(bucket, kind="stable")
    bs = bucket[order]
    counts = np.bincount(bucket, minlength=NCORES * NBLK)
    T = T_TILES
    maxc = int(counts.max())
    if maxc > T * P:
        T = 2 * (-(-maxc // (2 * P)))   # fallback: recompile, T kept even
    cap = T * P
    starts = np.zeros(NCORES * NBLK, dtype=np.int64)
    np.cumsum(counts[:-1], out=starts[1:])
    pos = np.arange(bs.size, dtype=np.int64) - starts[bs]
    src_pad = np.full((NCORES * NBLK, cap), ZERO_ROW, dtype=np.int32)
    dst_pad = np.zeros((NCORES * NBLK, cap), dtype=np.float32)
    src_pad[bs, pos] = allsrc[order].astype(np.int32)
    dst_pad[bs, pos] = within[order]

    # gather idxs, int16, wrapped in 16 partitions: idx i of a half-block
    # bucket lives at [i%16, i//16], replicated across the 8 Q7 cores'
    # 16-partition groups.
    cap2 = cap // 2
    seq = src_pad.reshape(NCORES, NBLK * 2, cap2 // 16, 16).astype(np.int16)
    seq = seq.transpose(0, 1, 3, 2)                 # [core, hb, 16, cap2/16]
    gidx = np.ascontiguousarray(
        np.broadcast_to(seq[:, :, None, :, :],
                        (NCORES, NBLK * 2, P // 16, 16, cap2 // 16))
        .reshape(NCORES, NBLK * 2, P, cap2 // 16))

    # per-tile int32 src ids (for the indirect-DMA fallback gather path)
    src_i32 = src_pad.reshape(NCORES, NBLK, T, P).transpose(0, 1, 3, 2)
    src_i32 = np.ascontiguousarray(src_i32)

    # one-hot dst ids: [core, blk, P, T, 1] bf16 (tile t at [., ., :, t, 0])
    dst_pad = dst_pad.reshape(NCORES, NBLK, T, P).transpose(0, 1, 3, 2)
    dst_pad = np.ascontiguousarray(dst_pad.astype(BF16))[..., None]

    colidx = np.broadcast_to(np.arange(P, dtype=np.float32)[None, :], (P, P))
    colidx = np.ascontiguousarray(colidx.astype(BF16))

    # W1 with b1 replicated into 128 extra columns (bias tile of Wf^T)
    w1b = np.concatenate(
        [W1, np.broadcast_to(b1[:, None], (MIDDLE, P))], axis=1)
    w1b_kxm = _pack(w1b.astype(BF16))                       # [P, 34, 640]
    w2t = W2.T.astype(BF16)                                 # [4352, 8192]

    in_maps = []
    for c in range(NCORES):
        vsl = slice(c * VSL, (c + 1) * VSL)
        in_maps.append({
            "x_rows": x_rows,
            "gidx": np.ascontiguousarray(gidx[c]),
            "src_ids": src_i32[c],
            "dst_ids": dst_pad[c],
            "colidx": colidx,
            "w1b_kxm": w1b_kxm,
            "w2t_kxn": _pack(np.ascontiguousarray(w2t[:, vsl])),
            "b2rep": np.ascontiguousarray(np.broadcast_to(
                b2[vsl].astype(BF16)[None, :], (P, VSL))),
        })
    return in_maps, T


def kernel(x, edge_index, W1, b1, W2, b2):
    global LAST_RESULTS

    in_maps, T = host_pack(x, edge_index, W1, b1, W2, b2)

    ins_dev, zeros = _prep_device_inputs(in_maps, T)
    outs = _run_once(ins_dev, zeros, T)
    LAST_RESULTS = dict(ins_dev=ins_dev, outs=outs, T=T)

    r = _make_runner(T)
    out_global = np.asarray(outs[r["out_names"].index("out_pvn")])
    out_global = out_global.reshape(NCORES, P, NBLK, VOCAB)

    out = np.empty((N_NODES, VOCAB), dtype=np.float32)
    for c in range(NCORES):
        o = _unpack(out_global[c])                          # [2560, 8192]
        out[c * ND:(c + 1) * ND] = o[:ND].astype(np.float32)
    return out


def bench(iters=5):
    """Steady-state per-call wall time (s); requires kernel() to have run."""
    import time
    import jax
    st = LAST_RESULTS
    assert st is not None, "run kernel() first"
    outs = st["outs"]
    times = []
    for _ in range(iters):
        t0 = time.perf_counter()
        outs = _run_once(st["ins_dev"], outs, st["T"])
        times.append(time.perf_counter() - t0)
    st["outs"] = outs
    return times


def bench_pipelined(iters=8):
    """Dispatch `iters` chained calls without blocking, block once.

    Successive calls are serialized on-device by the donated-output data
    dependence, while host dispatch overlaps — the per-iter slope is the
    device execution time.
    """
    import time
    import jax
    st = LAST_RESULTS
    assert st is not None, "run kernel() first"
    r = _make_runner(st["T"])
    outs = st["outs"]
    # warm: one blocked call so everything is resident
    outs = _run_once(st["ins_dev"], outs, st["T"])
    t0 = time.perf_counter()
    outs = r["fn"](*st["ins_dev"], *outs)
    jax.block_until_ready(outs)
    t1 = time.perf_counter() - t0
    t0 = time.perf_counter()
    for _ in range(iters):
        outs = r["fn"](*st["ins_dev"], *outs)
    jax.block_until_ready(outs)
    tN = time.perf_counter() - t0
    st["outs"] = outs
    per_iter = (tN - t1) / (iters - 1)
    return dict(t1=t1, tN=tN, iters=iters, per_iter=per_iter)


# revision 17
# speedup vs baseline: 1.6483x; 1.6483x over previous
"""GIN decoder (segment_sum aggregation + 2-layer linear MLP) on 8 trn2 cores.

Key optimizations vs the v1 kernel:
  * The decoder MLP has NO nonlinearity, so the two Linears fold into one:
    out = h @ (W2 @ W1).T + (b1 @ W2.T + b2).  ~9x fewer matmul FLOPs.
    Wf^T is computed on-device (vocab-sharded across the 8 cores, ~340
    matmuls/core) and AllGathered (DRAM->DRAM) while aggregation runs.
  * Gathers use InstDMAGatherAnt (nc.gpsimd.dma_gather): one instruction per
    2304 edges instead of one indirect DMA per 128 edges.  The old path paid
    ~1us SWDGE fixed cost per 128-row gather (795us of serialized gpsimd
    time); dma_gather generates descriptors via the Q7 counter machine
    (~0.34ns/descriptor) and spreads across 4 SWDGE queues.
  * One-hot scatter masks are built with a single batched is_equal per
    128-node block (bf16, 2x DVE rate) instead of 36 small ops.

Per-core schedule (SPMD, data-parallel over dst nodes):
  1. Wf^T slice: core c computes Wf^T[:, c*1024:(c+1)*1024] = sum_k
     W1[k,:]^T W2^T[k, vslice] on the PE, with an extra lhsT tile holding b1
     broadcast 128-wide so psum tile 4 evicts as (W2@b1 + b2) replicated
     across partitions.  Slices are AllGathered to every core.
  2. Aggregation: edges host-bucketed by (core, 128-wide dst block), padded
     to 36*128 per bucket with zero-row edges; each half-block dma_gathers
     2304 x-rows by src id and scatter-adds into its dst block via one-hot
     matmuls in PSUM; block results are PE-transposed to feature-major hT.
  3. Fused output matmul: out[128 nodes, 8192] = hT.T @ Wf^T + bias, K=512,
     bias added at PSUM evict, output written bf16 node-major.  Interleaved
     with aggregation so the PE stays busy while the AllGather completes.

All matmuls bf16 with f32 PSUM accumulation; output bf16 -> host f32.
"""

import numpy as np
import ml_dtypes

P = 128
N_NODES = 20000
HIDDEN = 512
MIDDLE = 4352
VOCAB = 8192
NCORES = 8
ND = N_NODES // NCORES          # 2500 nodes per core
NDP = 2560                      # padded to 20*128
KSRC = 20096                    # 157*128, gather source rows padded
BF16 = ml_dtypes.bfloat16

NBLK = NDP // P                 # 20 dst blocks of 128 per core
T_TILES = 36                    # 128-edge tiles per dst block (4608 cap)
ZERO_ROW = N_NODES              # gather target row holding zeros

VSL = VOCAB // NCORES           # 1024 vocab cols of Wf computed per core
KT = MIDDLE // P                # 34 contraction tiles for the Wf fold
FT = HIDDEN // P                # 4 feature tiles
MT_W = FT + 1                   # 4 feat tiles + 1 bias tile in Wf^T slice
NVT = VOCAB // 512              # 16 vocab tiles in fused matmul

CC_AFTER_BLOCK = 6              # emit AllGather after this many agg blocks
DELAY = 8                       # agg blocks in flight before first fused mm
NSWQ = 1                        # SWDGE queues for gather round-robin
USE_DMA_GATHER = True           # bisect flag: False -> per-tile indirect DMAs

_BUILT = {}
LAST_RESULTS = None             # state of the last run (for test.py)


def _pack(a):
    """[K, M] row-major -> partition-tiled [P, K//P, M] (row r -> [r%P, r//P, :])."""
    K, M = a.shape
    assert K % P == 0, (K, M)
    return np.ascontiguousarray(a.reshape(K // P, P, M).transpose(1, 0, 2))


def _unpack(a):
    """[P, MB, N] -> [MB*P, N]."""
    Pp, MB, N = a.shape
    return np.ascontiguousarray(a.transpose(1, 0, 2)).reshape(MB * Pp, N)


def _build(T=T_TILES):
    key = ("nc", T)
    if key in _BUILT:
        return _BUILT[key]
    from contextlib import ExitStack
    from concourse import bacc, mybir
    import concourse.bass as bass
    import concourse.tile as tile
    from concourse.masks import make_identity

    TH = T // 2                 # tiles per half-block gather
    NIDX = TH * P               # idxs per dma_gather (2304)

    dt = mybir.dt
    nc = bacc.Bacc("TRN2", target_bir_lowering=False, debug=False,
                   num_devices=NCORES, num_swdge_queues=NSWQ)

    x_rows = nc.dram_tensor("x_rows", [KSRC, HIDDEN], dt.bfloat16,
                            kind="ExternalInput").ap()
    gidx = nc.dram_tensor("gidx", [NBLK * 2, P, NIDX // 16], dt.int16,
                          kind="ExternalInput").ap()
    src_ids = nc.dram_tensor("src_ids", [NBLK, P, T], dt.int32,
                             kind="ExternalInput").ap()
    dst_ids = nc.dram_tensor("dst_ids", [NBLK, P, T, 1], dt.bfloat16,
                             kind="ExternalInput").ap()
    colidx = nc.dram_tensor("colidx", [P, P], dt.bfloat16,
                            kind="ExternalInput").ap()
    w1b_kxm = nc.dram_tensor("w1b_kxm", [P, KT, HIDDEN + P], dt.bfloat16,
                             kind="ExternalInput").ap()
    w2t_kxn = nc.dram_tensor("w2t_kxn", [P, KT, VSL], dt.bfloat16,
                             kind="ExternalInput").ap()
    b2rep = nc.dram_tensor("b2rep", [P, VSL], dt.bfloat16,
                           kind="ExternalInput").ap()
    out_pvn = nc.dram_tensor("out_pvn", [P, NBLK, VOCAB], dt.bfloat16,
                             kind="ExternalOutput").ap()

    with tile.TileContext(nc) as tc:
        with ExitStack() as ctx:
            dram = ctx.enter_context(tc.tile_pool(name="dram", bufs=1,
                                                  space="DRAM"))
            wf_slice = dram.tile([P, MT_W, VSL], dt.bfloat16)
            wf_all = dram.tile([NCORES, P, MT_W, VSL], dt.bfloat16,
                               addr_space="Shared")

            const = ctx.enter_context(tc.tile_pool(name="const", bufs=1))
            idp = ctx.enter_context(tc.tile_pool(name="idx", bufs=1))

            # index/one-hot operand loads ride the scalar (ACT) HWDGE queue
            # so they don't delay the weight loads on the sync queue
            def load_ids(b):
                gi = []
                for hb in range(2):
                    g = idp.tile([P, NIDX // 16], dt.int16,
                                 name=f"gidx{2 * b + hb}")
                    nc.scalar.dma_start(g[:], gidx[2 * b + hb])
                    gi.append(g)
                did = idp.tile([P, T, 1], dt.bfloat16, name=f"did{b}")
                nc.scalar.dma_start(did[:], dst_ids[b])
                sid = None
                if not USE_DMA_GATHER:
                    sid = idp.tile([P, T], dt.int32, name=f"sid{b}")
                    nc.scalar.dma_start(sid[:], src_ids[b])
                return gi, did, sid

            ids = [load_ids(b) for b in range(NBLK)]

            colidx_sb = const.tile([P, 1, P], dt.bfloat16)
            nc.sync.dma_start(colidx_sb[:, 0, :], colidx[:])
            ident = const.tile([P, P], dt.float32)
            make_identity(nc, ident[:])

            # agg-phase pools allocated BEFORE the Wf weight pools so the
            # gather buffers don't alias the Wf weights' SBUF space (aliasing
            # would stall the first gathers until every Wf matmul retires).
            hp = ctx.enter_context(tc.tile_pool(name="hT", bufs=DELAY + 2))
            gp = ctx.enter_context(tc.tile_pool(
                name="gath", bufs=2 if USE_DMA_GATHER else 12))
            ohp = ctx.enter_context(tc.tile_pool(name="oneh", bufs=2))
            hsbp = ctx.enter_context(tc.tile_pool(name="hsb", bufs=2))
            outp = ctx.enter_context(tc.tile_pool(name="outp", bufs=4))

            # ---- Phase 1: Wf^T slice = W1(+b1) x W2^T[:, vslice] ----
            # Bias tile (mt=4) emitted first so its DVE bias-add doesn't
            # queue behind the whole Wf phase and stall the agg one-hots.
            with ExitStack() as pctx:
                wpool = pctx.enter_context(tc.tile_pool(name="wf_w", bufs=1))
                w1b_sb = wpool.tile([P, KT, HIDDEN + P], dt.bfloat16)
                nc.sync.dma_start(w1b_sb[:], w1b_kxm[:])
                w2t_sb = wpool.tile([P, KT, VSL], dt.bfloat16)
                nc.sync.dma_start(w2t_sb[:], w2t_kxn[:])
                b2_sb = wpool.tile([P, VSL], dt.bfloat16)
                nc.sync.dma_start(b2_sb[:], b2rep[:])
                wfps = pctx.enter_context(tc.tile_pool(name="wf_ps", bufs=2,
                                                       space="PSUM"))
                wev = pctx.enter_context(tc.tile_pool(name="wf_ev", bufs=3))
                for mt in [MT_W - 1] + list(range(MT_W - 1)):
                    for v in range(VSL // 512):
                        ps = wfps.tile([P, 512], dt.float32, space="PSUM")
                        for kt in range(KT):
                            nc.tensor.matmul(
                                ps[:],
                                lhsT=w1b_sb[:, kt, mt * P:(mt + 1) * P],
                                rhs=w2t_sb[:, kt, v * 512:(v + 1) * 512],
                                start=(kt == 0), stop=(kt == KT - 1))
                        ev = wev.tile([P, 512], dt.bfloat16)
                        if mt == MT_W - 1:
                            nc.vector.tensor_tensor(
                                out=ev[:], in0=ps[:],
                                in1=b2_sb[:, v * 512:(v + 1) * 512],
                                op=mybir.AluOpType.add)
                        else:
                            nc.scalar.copy(ev[:], ps[:])
                        nc.sync.dma_start(
                            wf_slice[:, mt, v * 512:(v + 1) * 512], ev[:])

            # ---- Phase 2+3: aggregation + fused matmul, pipelined ----
            # wf_sb reuses the SBUF space of the (now-freed) Wf weight pool;
            # its writes depend on the AllGather anyway.
            main = ctx.enter_context(tc.tile_pool(name="main", bufs=1))
            wf_sb = main.tile([P, MT_W, VOCAB], dt.bfloat16)
            aggps = ctx.enter_context(tc.tile_pool(name="agg_ps", bufs=2,
                                                   space="PSUM"))
            tpps = ctx.enter_context(tc.tile_pool(name="tp_ps", bufs=2,
                                                  space="PSUM"))
            fps = ctx.enter_context(tc.tile_pool(name="f_ps", bufs=3,
                                                 space="PSUM"))

            hts = []

            def agg_block(b):
                gi, did, sid = ids[b]
                oh = ohp.tile([P, T, P], dt.bfloat16, name="onehot")
                nc.vector.tensor_tensor(
                    out=oh[:], in0=did[:].to_broadcast([P, T, P]),
                    in1=colidx_sb[:].to_broadcast([P, T, P]),
                    op=mybir.AluOpType.is_equal)
                ps = aggps.tile([P, HIDDEN], dt.float32, space="PSUM")
                if USE_DMA_GATHER:
                    gs = []
                    for hb in range(2):
                        g = gp.tile([P, TH, HIDDEN], dt.bfloat16, name="gather")
                        nc.gpsimd.dma_gather(
                            g[:], x_rows[:], gi[hb][:], NIDX, NIDX, HIDDEN,
                            single_packet=False,
                            queue_num=(2 * b + hb) % NSWQ)
                        gs.append(g)
                    for t in range(T):
                        nc.tensor.matmul(ps[:], lhsT=oh[:, t, :],
                                         rhs=gs[t // TH][:, t % TH, :],
                                         start=(t == 0), stop=(t == T - 1))
                else:
                    for t in range(T):
                        g = gp.tile([P, HIDDEN], dt.bfloat16, name="gather")
                        nc.gpsimd.indirect_dma_start(
                            out=g[:], out_offset=None, in_=x_rows[:],
                            in_offset=bass.IndirectOffsetOnAxis(
                                ap=sid[:, t:t + 1], axis=0))
                        nc.tensor.matmul(ps[:], lhsT=oh[:, t, :],
                                         rhs=g[:],
                                         start=(t == 0), stop=(t == T - 1))
                hsb = hsbp.tile([P, HIDDEN], dt.float32, name="hsb")
                nc.scalar.copy(hsb[:], ps[:])
                ht = hp.tile([P, FT, P], dt.bfloat16, name="ht")
                for j in range(FT):
                    tp = tpps.tile([P, P], dt.float32, space="PSUM")
                    nc.tensor.transpose(out=tp[:], in_=hsb[:, j * P:(j + 1) * P],
                                        identity=ident[:])
                    nc.vector.tensor_copy(ht[:, j, :], tp[:])
                hts.append(ht)

            def fused_block(b):
                ht = hts[b]
                for v in range(NVT):
                    ps = fps.tile([P, 512], dt.float32, space="PSUM")
                    for k in range(FT):
                        nc.tensor.matmul(
                            ps[:], lhsT=ht[:, k, :],
                            rhs=wf_sb[:, k, v * 512:(v + 1) * 512],
                            start=(k == 0), stop=(k == FT - 1))
                    ob = outp.tile([P, 512], dt.bfloat16, name="ob")
                    nc.vector.tensor_tensor(
                        out=ob[:], in0=ps[:],
                        in1=wf_sb[:, MT_W - 1, v * 512:(v + 1) * 512],
                        op=mybir.AluOpType.add)
                    nc.sync.dma_start(out_pvn[:, b, v * 512:(v + 1) * 512],
                                      ob[:])

            def emit_collective():
                nc.gpsimd.collective_compute(
                    "AllGather", mybir.AluOpType.bypass,
                    replica_groups=[list(range(NCORES))],
                    ins=[wf_slice[:].opt()],
                    outs=[wf_all[:].opt()])

            def emit_wf_load():
                for c in range(NCORES):
                    nc.sync.dma_start(wf_sb[:, :, c * VSL:(c + 1) * VSL],
                                      wf_all[c])

            for b in range(NBLK):
                agg_block(b)
                if b == CC_AFTER_BLOCK:
                    emit_collective()
                if b == DELAY - 1:
                    emit_wf_load()
                if b >= DELAY:
                    fused_block(b - DELAY)
            for b in range(NBLK - DELAY, NBLK):
                fused_block(b)

    nc.compile()
    _BUILT[key] = nc
    return nc


def _make_runner(T=T_TILES):
    """Build (once) a cached sharded-jit callable over the 8 cores.

    Returns dict with: fn(ins_dev, outs_prev) -> outs, names, avals, mesh,
    sharding.  Outputs are donated back in as the next call's (fully
    overwritten) output buffers, so steady-state calls move no host data.
    """
    rkey = ("runner", T)
    if rkey in _BUILT:
        return _BUILT[rkey]
    import jax
    from jax.experimental.shard_map import shard_map
    from jax.sharding import Mesh, NamedSharding, PartitionSpec
    from concourse import bass2jax, mybir

    nc = _build(T)
    bass2jax.install_neuronx_cc_hook()

    pid_name = (nc.partition_id_tensor.name
                if nc.partition_id_tensor is not None else None)
    in_names, out_names, out_avals = [], [], []
    for alloc in nc.m.functions[0].allocations:
        if not isinstance(alloc, mybir.MemoryLocationSet):
            continue
        name = alloc.memorylocations[0].name
        if alloc.kind == "ExternalInput":
            if name != pid_name:
                in_names.append(name)
        elif alloc.kind == "ExternalOutput":
            out_names.append(name)
            out_avals.append(jax.core.ShapedArray(
                tuple(alloc.tensor_shape), mybir.dt.np(alloc.dtype)))
    n_params = len(in_names)
    all_names = in_names + out_names
    if pid_name is not None:
        all_names = all_names + [pid_name]
    donate = tuple(range(n_params, n_params + len(out_names)))

    def _body(*args):
        operands = list(args)
        if pid_name is not None:
            operands.append(bass2jax.partition_id_tensor())
        outs = bass2jax._bass_exec_p.bind(
            *operands,
            out_avals=tuple(out_avals),
            in_names=tuple(all_names),
            out_names=tuple(out_names),
            lowering_input_output_aliases=(),
            sim_require_finite=True,
            sim_require_nnan=True,
            nc=nc,
        )
        return tuple(outs)

    devices = jax.devices()[:NCORES]
    mesh = Mesh(np.asarray(devices), ("core",))
    spec = PartitionSpec("core")
    in_specs = (spec,) * (n_params + len(out_names))
    out_specs = (spec,) * len(out_names)
    fn = jax.jit(
        shard_map(_body, mesh=mesh, in_specs=in_specs, out_specs=out_specs,
                  check_rep=False),
        donate_argnums=donate, keep_unused=True,
    )
    sharding = NamedSharding(mesh, spec)
    runner = dict(fn=fn, in_names=in_names, out_names=out_names,
                  out_avals=out_avals, sharding=sharding, mesh=mesh)
    _BUILT[rkey] = runner
    return runner


def _prep_device_inputs(in_maps, T=T_TILES):
    """device_put the concatenated per-core inputs; returns (ins_dev, zeros)."""
    import jax
    r = _make_runner(T)
    concat = [np.concatenate([m[name] for m in in_maps], axis=0)
              for name in r["in_names"]]
    ins_dev = [jax.device_put(a, r["sharding"]) for a in concat]
    zeros = [
        jax.jit(lambda a=av: jax.numpy.zeros(
            (NCORES * a.shape[0], *a.shape[1:]), a.dtype),
            out_shardings=r["sharding"])()
        for av in r["out_avals"]
    ]
    jax.block_until_ready(ins_dev + zeros)
    return ins_dev, zeros


def _run_once(ins_dev, out_bufs, T=T_TILES):
    import jax
    r = _make_runner(T)
    outs = r["fn"](*ins_dev, *out_bufs)
    jax.block_until_ready(outs)
    return outs


def host_pack(x, edge_index, W1, b1, W2, b2):
    """Host-side packing: returns (in_maps, T)."""
    x = np.asarray(x, dtype=np.float32)
    edge_index = np.asarray(edge_index)
    W1 = np.asarray(W1, dtype=np.float32)
    b1 = np.asarray(b1, dtype=np.float32)
    W2 = np.asarray(W2, dtype=np.float32)
    b2 = np.asarray(b2, dtype=np.float32)

    src = edge_index[0].astype(np.int64)
    dst = edge_index[1].astype(np.int64)

    # --- host packing (index preprocessing + layout/dtype shuffles) ---
    x_rows = np.zeros((KSRC, HIDDEN), dtype=BF16)
    x_rows[:N_NODES] = x

    # Edge list incl. self-loops (the GIN (1+eps)*x_i term, eps=0), bucketed
    # by (core, dst-block of 128) and padded to T*128 per bucket with edges
    # from the all-zeros row.
    allsrc = np.concatenate([src, np.arange(N_NODES, dtype=np.int64)])
    alldst = np.concatenate([dst, np.arange(N_NODES, dtype=np.int64)])
    core = alldst // ND
    local = alldst % ND
    blk = local // P
    within = (local % P).astype(np.int32)
    bucket = core * NBLK + blk
    order = np.argsort(bucket, kind="stable")
    bs = bucket[order]
    counts = np.bincount(bucket, minlength=NCORES * NBLK)
    T = T_TILES
    maxc = int(counts.max())
    if maxc > T * P:
        T = 2 * (-(-maxc // (2 * P)))   # fallback: recompile, T kept even
    cap = T * P
    starts = np.zeros(NCORES * NBLK, dtype=np.int64)
    np.cumsum(counts[:-1], out=starts[1:])
    pos = np.arange(bs.size, dtype=np.int64) - starts[bs]
    src_pad = np.full((NCORES * NBLK, cap), ZERO_ROW, dtype=np.int32)
    dst_pad = np.zeros((NCORES * NBLK, cap), dtype=np.float32)
    src_pad[bs, pos] = allsrc[order].astype(np.int32)
    dst_pad[bs, pos] = within[order]

    # gather idxs, int16, wrapped in 16 partitions: idx i of a half-block
    # bucket lives at [i%16, i//16], replicated across the 8 Q7 cores'
    # 16-partition groups.
    cap2 = cap // 2
    seq = src_pad.reshape(NCORES, NBLK * 2, cap2 // 16, 16).astype(np.int16)
    seq = seq.transpose(0, 1, 3, 2)                 # [core, hb, 16, cap2/16]
    gidx = np.ascontiguousarray(
        np.broadcast_to(seq[:, :, None, :, :],
                        (NCORES, NBLK * 2, P // 16, 16, cap2 // 16))
        .reshape(NCORES, NBLK * 2, P, cap2 // 16))

    # per-tile int32 src ids (for the indirect-DMA fallback gather path)
    src_i32 = src_pad.reshape(NCORES, NBLK, T, P).transpose(0, 1, 3, 2)
    src_i32 = np.ascontiguousarray(src_i32)

    # one-hot dst ids: [core, blk, P, T, 1] bf16 (tile t at [., ., :, t, 0])
    dst_pad = dst_pad.reshape(NCORES, NBLK, T, P).transpose(0, 1, 3, 2)
    dst_pad = np.ascontiguousarray(dst_pad.astype(BF16))[..., None]

    colidx = np.broadcast_to(np.arange(P, dtype=np.float32)[None, :], (P, P))
    colidx = np.ascontiguousarray(colidx.astype(BF16))

    # W1 with b1 replicated into 128 extra columns (bias tile of Wf^T)
    w1b = np.concatenate(
        [W1, np.broadcast_to(b1[:, None], (MIDDLE, P))], axis=1)
    w1b_kxm = _pack(w1b.astype(BF16))                       # [P, 34, 640]
    w2t = W2.T.astype(BF16)                                 # [4352, 8192]

    in_maps = []
    for c in range(NCORES):
        vsl = slice(c * VSL, (c + 1) * VSL)
        in_maps.append({
            "x_rows": x_rows,
            "gidx": np.ascontiguousarray(gidx[c]),
            "src_ids": src_i32[c],
            "dst_ids": dst_pad[c],
            "colidx": colidx,
            "w1b_kxm": w1b_kxm,
            "w2t_kxn": _pack(np.ascontiguousarray(w2t[:, vsl])),
            "b2rep": np.ascontiguousarray(np.broadcast_to(
                b2[vsl].astype(BF16)[None, :], (P, VSL))),
        })
    return in_maps, T


def kernel(x, edge_index, W1, b1, W2, b2):
    global LAST_RESULTS

    in_maps, T = host_pack(x, edge_index, W1, b1, W2, b2)

    ins_dev, zeros = _prep_device_inputs(in_maps, T)
    outs = _run_once(ins_dev, zeros, T)
    LAST_RESULTS = dict(ins_dev=ins_dev, outs=outs, T=T)

    r = _make_runner(T)
    out_global = np.asarray(outs[r["out_names"].index("out_pvn")])
    out_global = out_global.reshape(NCORES, P, NBLK, VOCAB)

    out = np.empty((N_NODES, VOCAB), dtype=np.float32)
    for c in range(NCORES):
        o = _unpack(out_global[c])                          # [2560, 8192]
        out[c * ND:(c + 1) * ND] = o[:ND].astype(np.float32)
    return out


def bench(iters=5):
    """Steady-state per-call wall time (s); requires kernel() to have run."""
    import time
    import jax
    st = LAST_RESULTS
    assert st is not None, "run kernel() first"
    outs = st["outs"]
    times = []
    for _ in range(iters):
        t0 = time.perf_counter()
        outs = _run_once(st["ins_dev"], outs, st["T"])
        times.append(time.perf_counter() - t0)
    st["outs"] = outs
    return times


def bench_pipelined(iters=8):
    """Dispatch `iters` chained calls without blocking, block once.

    Successive calls are serialized on-device by the donated-output data
    dependence, while host dispatch overlaps — the per-iter slope is the
    device execution time.
    """
    import time
    import jax
    st = LAST_RESULTS
    assert st is not None, "run kernel() first"
    r = _make_runner(st["T"])
    outs = st["outs"]
    # warm: one blocked call so everything is resident
    outs = _run_once(st["ins_dev"], outs, st["T"])
    t0 = time.perf_counter()
    outs = r["fn"](*st["ins_dev"], *outs)
    jax.block_until_ready(outs)
    t1 = time.perf_counter() - t0
    t0 = time.perf_counter()
    for _ in range(iters):
        outs = r["fn"](*st["ins_dev"], *outs)
    jax.block_until_ready(outs)
    tN = time.perf_counter() - t0
    st["outs"] = outs
    per_iter = (tN - t1) / (iters - 1)
    return dict(t1=t1, tN=tN, iters=iters, per_iter=per_iter)


# revision 28
# speedup vs baseline: 3.0658x; 1.8600x over previous
"""GIN decoder (segment_sum aggregation + 2-layer linear MLP) on 8 trn2 cores.

Key optimizations vs the v1 kernel:
  * The decoder MLP has NO nonlinearity, so the two Linears fold into one:
    out = h @ (W2 @ W1).T + (b1 @ W2.T + b2).  ~9x fewer matmul FLOPs.
    Wf^T is computed on-device (vocab-sharded across the 8 cores, ~340
    matmuls/core) and AllGathered (DRAM->DRAM) while aggregation runs.
  * Gathers use InstDMAGatherAnt (nc.gpsimd.dma_gather): one instruction per
    2304 edges instead of one indirect DMA per 128 edges.  The old path paid
    ~1us SWDGE fixed cost per 128-row gather (795us of serialized gpsimd
    time); dma_gather generates descriptors via the Q7 counter machine
    (~0.34ns/descriptor) and spreads across 4 SWDGE queues.
  * One-hot scatter masks are built with a single batched is_equal per
    128-node block (bf16, 2x DVE rate) instead of 36 small ops.

Per-core schedule (SPMD, data-parallel over dst nodes):
  1. Wf^T slice: core c computes Wf^T[:, c*1024:(c+1)*1024] = sum_k
     W1[k,:]^T W2^T[k, vslice] on the PE, with an extra lhsT tile holding b1
     broadcast 128-wide so psum tile 4 evicts as (W2@b1 + b2) replicated
     across partitions.  Slices are AllGathered to every core.
  2. Aggregation: edges host-bucketed by (core, 128-wide dst block), padded
     to 36*128 per bucket with zero-row edges; each half-block dma_gathers
     2304 x-rows by src id and scatter-adds into its dst block via one-hot
     matmuls in PSUM; block results are PE-transposed to feature-major hT.
  3. Fused output matmul: out[128 nodes, 8192] = hT.T @ Wf^T + bias, K=512,
     bias added at PSUM evict, output written bf16 node-major.  Interleaved
     with aggregation so the PE stays busy while the AllGather completes.

All matmuls bf16 with f32 PSUM accumulation; output bf16 -> host f32.
"""

import numpy as np
import ml_dtypes

P = 128
N_NODES = 20000
HIDDEN = 512
MIDDLE = 4352
VOCAB = 8192
NCORES = 8
ND = N_NODES // NCORES          # 2500 nodes per core
NDP = 2560                      # padded to 20*128
KSRC = 20096                    # 157*128, gather source rows padded
BF16 = ml_dtypes.bfloat16

NBLK = NDP // P                 # 20 dst blocks of 128 per core
T_TILES = 36                    # 128-edge tiles per dst block (4608 cap)
ZERO_ROW = N_NODES              # gather target row holding zeros

VSL = VOCAB // NCORES           # 1024 vocab cols of Wf computed per core
KT = MIDDLE // P                # 34 contraction tiles for the Wf fold
FT = HIDDEN // P                # 4 feature tiles
MT_W = FT + 1                   # 4 feat tiles + 1 bias tile in Wf^T slice
NVT = VOCAB // 512              # 16 vocab tiles in fused matmul

CC_AFTER_BLOCK = 3              # emit AllGather after this many agg blocks
DELAY = 8                       # agg blocks in flight before first fused mm
NSWQ = 4                        # SWDGE queues for gather round-robin
USE_DMA_GATHER = True           # bisect flag: False -> per-tile indirect DMAs

_BUILT = {}
LAST_RESULTS = None             # state of the last run (for test.py)


def _pack(a):
    """[K, M] row-major -> partition-tiled [P, K//P, M] (row r -> [r%P, r//P, :])."""
    K, M = a.shape
    assert K % P == 0, (K, M)
    return np.ascontiguousarray(a.reshape(K // P, P, M).transpose(1, 0, 2))


def _unpack(a):
    """[P, MB, N] -> [MB*P, N]."""
    Pp, MB, N = a.shape
    return np.ascontiguousarray(a.transpose(1, 0, 2)).reshape(MB * Pp, N)


def _build(T=T_TILES):
    key = ("nc", T)
    if key in _BUILT:
        return _BUILT[key]
    from contextlib import ExitStack
    from concourse import bacc, mybir
    import concourse.bass as bass
    import concourse.tile as tile
    from concourse.masks import make_identity

    NGB = 6                     # gathers per block
    TQ = T // NGB               # tiles per gather
    NIDX = TQ * P               # idxs per dma_gather (768)

    dt = mybir.dt
    nc = bacc.Bacc("TRN2", target_bir_lowering=False, debug=False,
                   num_devices=NCORES, num_swdge_queues=NSWQ)

    x_rows = nc.dram_tensor("x_rows", [KSRC, HIDDEN], dt.bfloat16,
                            kind="ExternalInput").ap()
    gidx = nc.dram_tensor("gidx", [NBLK, P, NGB, NIDX // 16], dt.int16,
                          kind="ExternalInput").ap()
    src_ids = nc.dram_tensor("src_ids", [NBLK, P, T], dt.int32,
                             kind="ExternalInput").ap()
    dst_ids = nc.dram_tensor("dst_ids", [NBLK, P, T, 1], dt.bfloat16,
                             kind="ExternalInput").ap()
    colidx = nc.dram_tensor("colidx", [P, P], dt.bfloat16,
                            kind="ExternalInput").ap()
    w1b_kxm = nc.dram_tensor("w1b_kxm", [P, KT, HIDDEN + P], dt.bfloat16,
                             kind="ExternalInput").ap()
    w2t_kxn = nc.dram_tensor("w2t_kxn", [P, KT, VSL], dt.bfloat16,
                             kind="ExternalInput").ap()
    b2rep = nc.dram_tensor("b2rep", [P, VSL], dt.bfloat16,
                           kind="ExternalInput").ap()
    out_pvn = nc.dram_tensor("out_pvn", [P, NBLK, VOCAB], dt.bfloat16,
                             kind="ExternalOutput").ap()

    with tile.TileContext(nc) as tc:
        with ExitStack() as ctx:
            dram = ctx.enter_context(tc.tile_pool(name="dram", bufs=1,
                                                  space="DRAM"))
            wf_slice = dram.tile([P, MT_W, VSL], dt.bfloat16)
            wf_all = dram.tile([NCORES, P, MT_W, VSL], dt.bfloat16,
                               addr_space="Shared")

            const = ctx.enter_context(tc.tile_pool(name="const", bufs=1))
            idp = ctx.enter_context(tc.tile_pool(name="idx", bufs=1))

            # Wf fold operands load first (w1b on sync, w2t chunk 0 + b2 on
            # scalar) so the fold starts as early as possible; the id/one-hot
            # loads follow on the scalar queue.
            def load_ids(b):
                gt = idp.tile([P, NGB, NIDX // 16], dt.int16,
                              name=f"gidx{b}")
                nc.sync.dma_start(gt[:], gidx[b])
                gi = [gt[:, qb, :] for qb in range(NGB)]
                did = idp.tile([P, T, 1], dt.bfloat16, name=f"did{b}")
                nc.sync.dma_start(did[:], dst_ids[b])
                sid = None
                if not USE_DMA_GATHER:
                    sid = idp.tile([P, T], dt.int32, name=f"sid{b}")
                    nc.sync.dma_start(sid[:], src_ids[b])
                return gi, did, sid

            colidx_sb = const.tile([P, 1, P], dt.bfloat16)
            nc.sync.dma_start(colidx_sb[:, 0, :], colidx[:])
            ident = const.tile([P, P], dt.float32)
            make_identity(nc, ident[:])
            ones_sb = const.tile([1, P], dt.bfloat16)
            nc.vector.memset(ones_sb[:], 1.0)

            # agg-phase pools allocated BEFORE the Wf weight pools so the
            # gather buffers don't alias the Wf weights' SBUF space (aliasing
            # would stall the first gathers until every Wf matmul retires).
            hp = ctx.enter_context(tc.tile_pool(name="hT", bufs=DELAY + 1))
            gp = ctx.enter_context(tc.tile_pool(
                name="gath", bufs=8 if USE_DMA_GATHER else 12))
            ohp = ctx.enter_context(tc.tile_pool(name="oneh", bufs=2))
            hsbp = ctx.enter_context(tc.tile_pool(name="hsb", bufs=2))
            outp = ctx.enter_context(tc.tile_pool(name="outp", bufs=3))

            # Wf fold operands: created last so they can be freed (LIFO)
            # right after phase 1; loads go out first on their queues.
            wctx = ExitStack()
            wpool0 = wctx.enter_context(tc.tile_pool(name="wf_w0", bufs=1))
            w1b_sb = wpool0.tile([P, KT, HIDDEN + P], dt.bfloat16)
            nc.sync.dma_start(w1b_sb[:], w1b_kxm[:])
            w2p = wctx.enter_context(tc.tile_pool(name="wf_w2", bufs=1))
            w2t_sb = w2p.tile([P, KT, VSL], dt.bfloat16)
            nc.scalar.dma_start(w2t_sb[:], w2t_kxn[:])
            b2_sb = wpool0.tile([P, VSL], dt.bfloat16)
            nc.scalar.dma_start(b2_sb[:], b2rep[:])

            ids = [load_ids(b) for b in range(NBLK)]

            # ---- Phase 1: Wf^T slice = W1(+b1) x W2^T[:, vslice] ----
            # Bias tile (mt=4) emitted first so its DVE bias-add doesn't
            # queue behind the whole Wf phase and stall the agg one-hots.
            with ExitStack() as pctx:
                wfps = pctx.enter_context(tc.tile_pool(name="wf_ps", bufs=2,
                                                       space="PSUM"))
                wev = pctx.enter_context(tc.tile_pool(name="wf_ev", bufs=2))
                for v in range(VSL // 512):
                    for mt in [MT_W - 1] + list(range(MT_W - 1)):
                        ps = wfps.tile([P, 512], dt.float32, space="PSUM")
                        for kt in range(KT):
                            nc.tensor.matmul(
                                ps[:],
                                lhsT=w1b_sb[:, kt, mt * P:(mt + 1) * P],
                                rhs=w2t_sb[:, kt, v * 512:(v + 1) * 512],
                                start=(kt == 0), stop=(kt == KT - 1))
                        ev = wev.tile([P, 512], dt.bfloat16)
                        if mt == MT_W - 1:
                            nc.vector.tensor_tensor(
                                out=ev[:], in0=ps[:],
                                in1=b2_sb[:, v * 512:(v + 1) * 512],
                                op=mybir.AluOpType.add)
                        else:
                            nc.vector.tensor_copy(ev[:], ps[:])
                        nc.sync.dma_start(
                            wf_slice[:, mt, v * 512:(v + 1) * 512], ev[:])

            wctx.close()

            # ---- Phase 2+3: aggregation + fused matmul, pipelined ----
            # wf_sb reuses the SBUF space of the (now-freed) Wf weight pool;
            # its writes depend on the AllGather anyway.
            main = ctx.enter_context(tc.tile_pool(name="main", bufs=1))
            wf_sb = main.tile([P, MT_W, VOCAB], dt.bfloat16)
            aggps = ctx.enter_context(tc.tile_pool(name="agg_ps", bufs=2,
                                                   space="PSUM"))
            tpps = ctx.enter_context(tc.tile_pool(name="tp_ps", bufs=2,
                                                  space="PSUM"))
            fps = ctx.enter_context(tc.tile_pool(name="f_ps", bufs=3,
                                                 space="PSUM"))

            hts = []

            def agg_block(b):
                gi, did, sid = ids[b]
                ohs = []
                for hh in range(2):
                    oh = ohp.tile([P, T // 2, P], dt.bfloat16, name="onehot")
                    nc.vector.tensor_tensor(
                        out=oh[:],
                        in0=did[:, hh * (T // 2):(hh + 1) * (T // 2), :]
                            .to_broadcast([P, T // 2, P]),
                        in1=colidx_sb[:].to_broadcast([P, T // 2, P]),
                        op=mybir.AluOpType.is_equal)
                    ohs.append(oh)
                ps = aggps.tile([P, HIDDEN], dt.float32, space="PSUM")
                if USE_DMA_GATHER:
                    gs = []
                    for qb in range(NGB):
                        g = gp.tile([P, TQ, HIDDEN], dt.bfloat16, name="gather")
                        nc.gpsimd.dma_gather(
                            g[:], x_rows[:], gi[qb], NIDX, NIDX, HIDDEN,
                            single_packet=False,
                            queue_num=(NGB * b + qb) % NSWQ)
                        gs.append(g)
                    for t in range(T):
                        nc.tensor.matmul(ps[:],
                                         lhsT=ohs[t // (T // 2)][:, t % (T // 2), :],
                                         rhs=gs[t // TQ][:, t % TQ, :],
                                         start=(t == 0), stop=(t == T - 1))
                else:
                    for t in range(T):
                        g = gp.tile([P, HIDDEN], dt.bfloat16, name="gather")
                        nc.gpsimd.indirect_dma_start(
                            out=g[:], out_offset=None, in_=x_rows[:],
                            in_offset=bass.IndirectOffsetOnAxis(
                                ap=sid[:, t:t + 1], axis=0))
                        nc.tensor.matmul(ps[:],
                                         lhsT=ohs[t // (T // 2)][:, t % (T // 2), :],
                                         rhs=g[:],
                                         start=(t == 0), stop=(t == T - 1))
                hsb = hsbp.tile([P, HIDDEN], dt.float32, name="hsb")
                nc.scalar.copy(hsb[:], ps[:])
                ht = hp.tile([P, FT, P], dt.bfloat16, name="ht")
                for j in range(FT):
                    tp = tpps.tile([P, P], dt.float32, space="PSUM")
                    nc.tensor.transpose(out=tp[:], in_=hsb[:, j * P:(j + 1) * P],
                                        identity=ident[:])
                    nc.scalar.copy(ht[:, j, :], tp[:])
                hts.append(ht)

            def fused_block(b):
                ht = hts[b]
                for v in range(NVT):
                    ps = fps.tile([P, 512], dt.float32, space="PSUM")
                    nc.tensor.matmul(
                        ps[:], lhsT=ones_sb[:],
                        rhs=wf_sb[0:1, MT_W - 1, v * 512:(v + 1) * 512],
                        start=True, stop=False)
                    for k in range(FT):
                        nc.tensor.matmul(
                            ps[:], lhsT=ht[:, k, :],
                            rhs=wf_sb[:, k, v * 512:(v + 1) * 512],
                            start=False, stop=(k == FT - 1))
                    ob = outp.tile([P, 512], dt.bfloat16, name="ob")
                    nc.scalar.copy(ob[:], ps[:])
                    nc.sync.dma_start(out_pvn[:, b, v * 512:(v + 1) * 512],
                                      ob[:])

            def emit_collective():
                nc.gpsimd.collective_compute(
                    "AllGather", mybir.AluOpType.bypass,
                    replica_groups=[list(range(NCORES))],
                    ins=[wf_slice[:].opt()],
                    outs=[wf_all[:].opt()])

            def emit_wf_load():
                for c in range(NCORES):
                    nc.sync.dma_start(wf_sb[:, :, c * VSL:(c + 1) * VSL],
                                      wf_all[c])

            for b in range(NBLK):
                if b >= DELAY:
                    fused_block(b - DELAY)
                agg_block(b)
                if b == CC_AFTER_BLOCK:
                    emit_collective()
                if b == DELAY - 1:
                    emit_wf_load()
            for b in range(NBLK - DELAY, NBLK):
                fused_block(b)

    nc.compile()
    _BUILT[key] = nc
    return nc


def _make_runner(T=T_TILES):
    """Build (once) a cached sharded-jit callable over the 8 cores.

    Returns dict with: fn(ins_dev, outs_prev) -> outs, names, avals, mesh,
    sharding.  Outputs are donated back in as the next call's (fully
    overwritten) output buffers, so steady-state calls move no host data.
    """
    rkey = ("runner", T)
    if rkey in _BUILT:
        return _BUILT[rkey]
    import jax
    from jax.experimental.shard_map import shard_map
    from jax.sharding import Mesh, NamedSharding, PartitionSpec
    from concourse import bass2jax, mybir

    nc = _build(T)
    bass2jax.install_neuronx_cc_hook()

    pid_name = (nc.partition_id_tensor.name
                if nc.partition_id_tensor is not None else None)
    in_names, out_names, out_avals = [], [], []
    for alloc in nc.m.functions[0].allocations:
        if not isinstance(alloc, mybir.MemoryLocationSet):
            continue
        name = alloc.memorylocations[0].name
        if alloc.kind == "ExternalInput":
            if name != pid_name:
                in_names.append(name)
        elif alloc.kind == "ExternalOutput":
            out_names.append(name)
            out_avals.append(jax.core.ShapedArray(
                tuple(alloc.tensor_shape), mybir.dt.np(alloc.dtype)))
    n_params = len(in_names)
    all_names = in_names + out_names
    if pid_name is not None:
        all_names = all_names + [pid_name]
    donate = tuple(range(n_params, n_params + len(out_names)))

    def _body(*args):
        operands = list(args)
        if pid_name is not None:
            operands.append(bass2jax.partition_id_tensor())
        outs = bass2jax._bass_exec_p.bind(
            *operands,
            out_avals=tuple(out_avals),
            in_names=tuple(all_names),
            out_names=tuple(out_names),
            lowering_input_output_aliases=(),
            sim_require_finite=True,
            sim_require_nnan=True,
            nc=nc,
        )
        return tuple(outs)

    devices = jax.devices()[:NCORES]
    mesh = Mesh(np.asarray(devices), ("core",))
    spec = PartitionSpec("core")
    in_specs = (spec,) * (n_params + len(out_names))
    out_specs = (spec,) * len(out_names)
    fn = jax.jit(
        shard_map(_body, mesh=mesh, in_specs=in_specs, out_specs=out_specs,
                  check_rep=False),
        donate_argnums=donate, keep_unused=True,
    )
    sharding = NamedSharding(mesh, spec)
    runner = dict(fn=fn, in_names=in_names, out_names=out_names,
                  out_avals=out_avals, sharding=sharding, mesh=mesh)
    _BUILT[rkey] = runner
    return runner


def _prep_device_inputs(in_maps, T=T_TILES):
    """device_put the concatenated per-core inputs; returns (ins_dev, zeros)."""
    import jax
    r = _make_runner(T)
    concat = [np.concatenate([m[name] for m in in_maps], axis=0)
              for name in r["in_names"]]
    ins_dev = [jax.device_put(a, r["sharding"]) for a in concat]
    zeros = [
        jax.jit(lambda a=av: jax.numpy.zeros(
            (NCORES * a.shape[0], *a.shape[1:]), a.dtype),
            out_shardings=r["sharding"])()
        for av in r["out_avals"]
    ]
    jax.block_until_ready(ins_dev + zeros)
    return ins_dev, zeros


def _run_once(ins_dev, out_bufs, T=T_TILES):
    import jax
    r = _make_runner(T)
    outs = r["fn"](*ins_dev, *out_bufs)
    jax.block_until_ready(outs)
    return outs


def host_pack(x, edge_index, W1, b1, W2, b2):
    """Host-side packing: returns (in_maps, T)."""
    x = np.asarray(x, dtype=np.float32)
    edge_index = np.asarray(edge_index)
    W1 = np.asarray(W1, dtype=np.float32)
    b1 = np.asarray(b1, dtype=np.float32)
    W2 = np.asarray(W2, dtype=np.float32)
    b2 = np.asarray(b2, dtype=np.float32)

    src = edge_index[0].astype(np.int64)
    dst = edge_index[1].astype(np.int64)

    # --- host packing (index preprocessing + layout/dtype shuffles) ---
    x_rows = np.zeros((KSRC, HIDDEN), dtype=BF16)
    x_rows[:N_NODES] = x

    # Edge list incl. self-loops (the GIN (1+eps)*x_i term, eps=0), bucketed
    # by (core, dst-block of 128) and padded to T*128 per bucket with edges
    # from the all-zeros row.
    allsrc = np.concatenate([src, np.arange(N_NODES, dtype=np.int64)])
    alldst = np.concatenate([dst, np.arange(N_NODES, dtype=np.int64)])
    core = alldst // ND
    local = alldst % ND
    blk = local // P
    within = (local % P).astype(np.int32)
    bucket = core * NBLK + blk
    order = np.argsort(bucket, kind="stable")
    bs = bucket[order]
    counts = np.bincount(bucket, minlength=NCORES * NBLK)
    T = T_TILES
    maxc = int(counts.max())
    if maxc > T * P:
        T = 12 * (-(-maxc // (12 * P)))  # fallback: recompile, T /12 (6 gathers, 2 oh halves)
    cap = T * P
    starts = np.zeros(NCORES * NBLK, dtype=np.int64)
    np.cumsum(counts[:-1], out=starts[1:])
    pos = np.arange(bs.size, dtype=np.int64) - starts[bs]
    src_pad = np.full((NCORES * NBLK, cap), ZERO_ROW, dtype=np.int32)
    dst_pad = np.zeros((NCORES * NBLK, cap), dtype=np.float32)
    src_pad[bs, pos] = allsrc[order].astype(np.int32)
    dst_pad[bs, pos] = within[order]

    # gather idxs, int16, wrapped in 16 partitions: idx i of a half-block
    # bucket lives at [i%16, i//16], replicated across the 8 Q7 cores'
    # 16-partition groups.
    NGB = 6
    capq = cap // NGB
    seq = src_pad.reshape(NCORES, NBLK, NGB, capq // 16, 16).astype(np.int16)
    seq = seq.transpose(0, 1, 2, 4, 3)          # [core, b, qb, 16, capq/16]
    gidx = np.broadcast_to(
        seq[:, :, :, None, :, :],
        (NCORES, NBLK, NGB, P // 16, 16, capq // 16))
    gidx = np.ascontiguousarray(
        gidx.reshape(NCORES, NBLK, NGB, P, capq // 16).transpose(0, 1, 3, 2, 4))

    # per-tile int32 src ids (for the indirect-DMA fallback gather path)
    src_i32 = src_pad.reshape(NCORES, NBLK, T, P).transpose(0, 1, 3, 2)
    src_i32 = np.ascontiguousarray(src_i32)

    # one-hot dst ids: [core, blk, P, T, 1] bf16 (tile t at [., ., :, t, 0])
    dst_pad = dst_pad.reshape(NCORES, NBLK, T, P).transpose(0, 1, 3, 2)
    dst_pad = np.ascontiguousarray(dst_pad.astype(BF16))[..., None]

    colidx = np.broadcast_to(np.arange(P, dtype=np.float32)[None, :], (P, P))
    colidx = np.ascontiguousarray(colidx.astype(BF16))

    # W1 with b1 replicated into 128 extra columns (bias tile of Wf^T)
    w1b = np.concatenate(
        [W1, np.broadcast_to(b1[:, None], (MIDDLE, P))], axis=1)
    w1b_kxm = _pack(w1b.astype(BF16))                       # [P, 34, 640]
    w2t = W2.T.astype(BF16)                                 # [4352, 8192]

    in_maps = []
    for c in range(NCORES):
        vsl = slice(c * VSL, (c + 1) * VSL)
        in_maps.append({
            "x_rows": x_rows,
            "gidx": np.ascontiguousarray(gidx[c]),
            "src_ids": src_i32[c],
            "dst_ids": dst_pad[c],
            "colidx": colidx,
            "w1b_kxm": w1b_kxm,
            "w2t_kxn": _pack(np.ascontiguousarray(w2t[:, vsl])),
            "b2rep": np.ascontiguousarray(np.broadcast_to(
                b2[vsl].astype(BF16)[None, :], (P, VSL))),
        })
    return in_maps, T


def kernel(x, edge_index, W1, b1, W2, b2):
    global LAST_RESULTS

    in_maps, T = host_pack(x, edge_index, W1, b1, W2, b2)

    ins_dev, zeros = _prep_device_inputs(in_maps, T)
    outs = _run_once(ins_dev, zeros, T)
    LAST_RESULTS = dict(ins_dev=ins_dev, outs=outs, T=T)

    r = _make_runner(T)
    out_global = np.asarray(outs[r["out_names"].index("out_pvn")])
    out_global = out_global.reshape(NCORES, P, NBLK, VOCAB)

    out = np.empty((N_NODES, VOCAB), dtype=np.float32)
    for c in range(NCORES):
        o = _unpack(out_global[c])                          # [2560, 8192]
        out[c * ND:(c + 1) * ND] = o[:ND].astype(np.float32)
    return out


def bench(iters=5):
    """Steady-state per-call wall time (s); requires kernel() to have run."""
    import time
    import jax
    st = LAST_RESULTS
    assert st is not None, "run kernel() first"
    outs = st["outs"]
    times = []
    for _ in range(iters):
        t0 = time.perf_counter()
        outs = _run_once(st["ins_dev"], outs, st["T"])
        times.append(time.perf_counter() - t0)
    st["outs"] = outs
    return times


def bench_pipelined(iters=8):
    """Dispatch `iters` chained calls without blocking, block once.

    Successive calls are serialized on-device by the donated-output data
    dependence, while host dispatch overlaps — the per-iter slope is the
    device execution time.
    """
    import time
    import jax
    st = LAST_RESULTS
    assert st is not None, "run kernel() first"
    r = _make_runner(st["T"])
    outs = st["outs"]
    # warm: one blocked call so everything is resident
    outs = _run_once(st["ins_dev"], outs, st["T"])
    t0 = time.perf_counter()
    outs = r["fn"](*st["ins_dev"], *outs)
    jax.block_until_ready(outs)
    t1 = time.perf_counter() - t0
    t0 = time.perf_counter()
    for _ in range(iters):
        outs = r["fn"](*st["ins_dev"], *outs)
    jax.block_until_ready(outs)
    tN = time.perf_counter() - t0
    st["outs"] = outs
    per_iter = (tN - t1) / (iters - 1)
    return dict(t1=t1, tN=tN, iters=iters, per_iter=per_iter)


# revision 32
# speedup vs baseline: 3.2340x; 1.0549x over previous
"""GIN decoder (segment_sum aggregation + 2-layer linear MLP) on 8 trn2 cores.

Key optimizations vs the v1 kernel:
  * The decoder MLP has NO nonlinearity, so the two Linears fold into one:
    out = h @ (W2 @ W1).T + (b1 @ W2.T + b2).  ~9x fewer matmul FLOPs.
    Wf^T is computed on-device (vocab-sharded across the 8 cores, ~340
    matmuls/core) and AllGathered (DRAM->DRAM) while aggregation runs.
  * Gathers use InstDMAGatherAnt (nc.gpsimd.dma_gather): one instruction per
    2304 edges instead of one indirect DMA per 128 edges.  The old path paid
    ~1us SWDGE fixed cost per 128-row gather (795us of serialized gpsimd
    time); dma_gather generates descriptors via the Q7 counter machine
    (~0.34ns/descriptor) and spreads across 4 SWDGE queues.
  * One-hot scatter masks are built with a single batched is_equal per
    128-node block (bf16, 2x DVE rate) instead of 36 small ops.

Per-core schedule (SPMD, data-parallel over dst nodes):
  1. Wf^T slice: core c computes Wf^T[:, c*1024:(c+1)*1024] = sum_k
     W1[k,:]^T W2^T[k, vslice] on the PE, with an extra lhsT tile holding b1
     broadcast 128-wide so psum tile 4 evicts as (W2@b1 + b2) replicated
     across partitions.  Slices are AllGathered to every core.
  2. Aggregation: edges host-bucketed by (core, 128-wide dst block), padded
     to 36*128 per bucket with zero-row edges; each half-block dma_gathers
     2304 x-rows by src id and scatter-adds into its dst block via one-hot
     matmuls in PSUM; block results are PE-transposed to feature-major hT.
  3. Fused output matmul: out[128 nodes, 8192] = hT.T @ Wf^T + bias, K=512,
     bias added at PSUM evict, output written bf16 node-major.  Interleaved
     with aggregation so the PE stays busy while the AllGather completes.

All matmuls bf16 with f32 PSUM accumulation; output bf16 -> host f32.
"""

import numpy as np
import ml_dtypes

P = 128
N_NODES = 20000
HIDDEN = 512
MIDDLE = 4352
VOCAB = 8192
NCORES = 8
ND = N_NODES // NCORES          # 2500 nodes per core
NDP = 2560                      # padded to 20*128
KSRC = 20096                    # 157*128, gather source rows padded
BF16 = ml_dtypes.bfloat16

NBLK = NDP // P                 # 20 dst blocks of 128 per core
T_TILES = 36                    # 128-edge tiles per dst block (4608 cap)
ZERO_ROW = N_NODES              # gather target row holding zeros

VSL = VOCAB // NCORES           # 1024 vocab cols of Wf computed per core
KT = MIDDLE // P                # 34 contraction tiles for the Wf fold
FT = HIDDEN // P                # 4 feature tiles
MT_W = FT + 1                   # 4 feat tiles + 1 bias tile in Wf^T slice
NVT = VOCAB // 512              # 16 vocab tiles in fused matmul

CC_AFTER_BLOCK = 2              # emit AllGather after this many agg blocks
DELAY = 9                       # agg blocks in flight before first fused mm
NSWQ = 4                        # SWDGE queues for gather round-robin
USE_DMA_GATHER = True           # bisect flag: False -> per-tile indirect DMAs

_BUILT = {}
LAST_RESULTS = None             # state of the last run (for test.py)


def _pack(a):
    """[K, M] row-major -> partition-tiled [P, K//P, M] (row r -> [r%P, r//P, :])."""
    K, M = a.shape
    assert K % P == 0, (K, M)
    return np.ascontiguousarray(a.reshape(K // P, P, M).transpose(1, 0, 2))


def _unpack(a):
    """[P, MB, N] -> [MB*P, N]."""
    Pp, MB, N = a.shape
    return np.ascontiguousarray(a.transpose(1, 0, 2)).reshape(MB * Pp, N)


def _build(T=T_TILES, regs=None):
    key = ("nc", T, None if regs is None else tuple(map(tuple, regs)))
    if key in _BUILT:
        return _BUILT[key]
    if regs is None:
        regs = [[(T // 6) * P] * 6 for _ in range(NBLK)]
    from contextlib import ExitStack
    from concourse import bacc, mybir
    import concourse.bass as bass
    import concourse.tile as tile
    from concourse.masks import make_identity

    NGB = 6                     # gathers per block
    TQ = T // NGB               # tiles per gather
    NIDX = TQ * P               # idxs per dma_gather (768)

    dt = mybir.dt
    nc = bacc.Bacc("TRN2", target_bir_lowering=False, debug=False,
                   num_devices=NCORES, num_swdge_queues=NSWQ)

    x_rows = nc.dram_tensor("x_rows", [KSRC, HIDDEN], dt.bfloat16,
                            kind="ExternalInput").ap()
    gidx = nc.dram_tensor("gidx", [NBLK, P, NGB, NIDX // 16], dt.int16,
                          kind="ExternalInput").ap()
    src_ids = nc.dram_tensor("src_ids", [NBLK, P, T], dt.int32,
                             kind="ExternalInput").ap()
    dst_ids = nc.dram_tensor("dst_ids", [NBLK, P, T, 1], dt.bfloat16,
                             kind="ExternalInput").ap()
    colidx = nc.dram_tensor("colidx", [P, P], dt.bfloat16,
                            kind="ExternalInput").ap()
    w1b_kxm = nc.dram_tensor("w1b_kxm", [P, KT, HIDDEN + P], dt.bfloat16,
                             kind="ExternalInput").ap()
    w2t_kxn = nc.dram_tensor("w2t_kxn", [P, KT, VSL], dt.bfloat16,
                             kind="ExternalInput").ap()
    b2rep = nc.dram_tensor("b2rep", [P, VSL], dt.bfloat16,
                           kind="ExternalInput").ap()
    x_own = nc.dram_tensor("x_own", [NBLK, P, HIDDEN], dt.bfloat16,
                           kind="ExternalInput").ap()
    out_pvn = nc.dram_tensor("out_pvn", [P, NBLK, VOCAB], dt.bfloat16,
                             kind="ExternalOutput").ap()

    with tile.TileContext(nc) as tc:
        with ExitStack() as ctx:
            dram = ctx.enter_context(tc.tile_pool(name="dram", bufs=1,
                                                  space="DRAM"))
            wf_slice = dram.tile([P, MT_W, VSL], dt.bfloat16)
            wf_all = dram.tile([NCORES, P, MT_W, VSL], dt.bfloat16,
                               addr_space="Shared")

            const = ctx.enter_context(tc.tile_pool(name="const", bufs=1))
            idp = ctx.enter_context(tc.tile_pool(name="idx", bufs=1))

            # Wf fold operands load first (w1b on sync, w2t chunk 0 + b2 on
            # scalar) so the fold starts as early as possible; the id/one-hot
            # loads follow on the scalar queue.
            def load_ids(b):
                gt = idp.tile([P, NGB, NIDX // 16], dt.int16,
                              name=f"gidx{b}")
                nc.sync.dma_start(gt[:], gidx[b])
                gi = [gt[:, qb, :] for qb in range(NGB)]
                did = idp.tile([P, T, 1], dt.bfloat16, name=f"did{b}")
                nc.sync.dma_start(did[:], dst_ids[b])
                sid = None
                if not USE_DMA_GATHER:
                    sid = idp.tile([P, T], dt.int32, name=f"sid{b}")
                    nc.sync.dma_start(sid[:], src_ids[b])
                return gi, did, sid

            colidx_sb = const.tile([P, 1, P], dt.bfloat16)
            nc.sync.dma_start(colidx_sb[:, 0, :], colidx[:])
            ident = const.tile([P, P], dt.float32)
            make_identity(nc, ident[:])
            ones_sb = const.tile([1, P], dt.bfloat16)
            nc.vector.memset(ones_sb[:], 1.0)

            # agg-phase pools allocated BEFORE the Wf weight pools so the
            # gather buffers don't alias the Wf weights' SBUF space (aliasing
            # would stall the first gathers until every Wf matmul retires).
            hp = ctx.enter_context(tc.tile_pool(name="hT", bufs=DELAY + 1))
            gp = ctx.enter_context(tc.tile_pool(
                name="gath", bufs=8 if USE_DMA_GATHER else 12))
            ohp = ctx.enter_context(tc.tile_pool(name="oneh", bufs=2))
            hsbp = ctx.enter_context(tc.tile_pool(name="hsb", bufs=2))
            xop = ctx.enter_context(tc.tile_pool(name="xo", bufs=2))
            outp = ctx.enter_context(tc.tile_pool(name="outp", bufs=3))

            # Wf fold operands: created last so they can be freed (LIFO)
            # right after phase 1; loads go out first on their queues.
            wctx = ExitStack()
            wpool0 = wctx.enter_context(tc.tile_pool(name="wf_w0", bufs=1))
            w1b_sb = wpool0.tile([P, KT, HIDDEN + P], dt.bfloat16)
            nc.sync.dma_start(w1b_sb[:], w1b_kxm[:])
            w2p = wctx.enter_context(tc.tile_pool(name="wf_w2", bufs=1))
            w2t_sb = w2p.tile([P, KT, VSL], dt.bfloat16)
            nc.scalar.dma_start(w2t_sb[:], w2t_kxn[:])
            b2_sb = wpool0.tile([P, VSL], dt.bfloat16)
            nc.scalar.dma_start(b2_sb[:], b2rep[:])

            ids = [load_ids(b) for b in range(NBLK)]

            # ---- Phase 1: Wf^T slice = W1(+b1) x W2^T[:, vslice] ----
            # Bias tile (mt=4) emitted first so its DVE bias-add doesn't
            # queue behind the whole Wf phase and stall the agg one-hots.
            with ExitStack() as pctx:
                wfps = pctx.enter_context(tc.tile_pool(name="wf_ps", bufs=2,
                                                       space="PSUM"))
                wev = pctx.enter_context(tc.tile_pool(name="wf_ev", bufs=2))
                for v in range(VSL // 512):
                    for mt in [MT_W - 1] + list(range(MT_W - 1)):
                        ps = wfps.tile([P, 512], dt.float32, space="PSUM")
                        for kt in range(KT):
                            nc.tensor.matmul(
                                ps[:],
                                lhsT=w1b_sb[:, kt, mt * P:(mt + 1) * P],
                                rhs=w2t_sb[:, kt, v * 512:(v + 1) * 512],
                                start=(kt == 0), stop=(kt == KT - 1))
                        ev = wev.tile([P, 512], dt.bfloat16)
                        if mt == MT_W - 1:
                            nc.vector.tensor_tensor(
                                out=ev[:], in0=ps[:],
                                in1=b2_sb[:, v * 512:(v + 1) * 512],
                                op=mybir.AluOpType.add)
                        else:
                            nc.vector.tensor_copy(ev[:], ps[:])
                        nc.sync.dma_start(
                            wf_slice[:, mt, v * 512:(v + 1) * 512], ev[:])

            wctx.close()

            # ---- Phase 2+3: aggregation + fused matmul, pipelined ----
            # wf_sb reuses the SBUF space of the (now-freed) Wf weight pool;
            # its writes depend on the AllGather anyway.
            main = ctx.enter_context(tc.tile_pool(name="main", bufs=1))
            wf_sb = main.tile([P, MT_W, VOCAB], dt.bfloat16)
            aggps = ctx.enter_context(tc.tile_pool(name="agg_ps", bufs=2,
                                                   space="PSUM"))
            tpps = ctx.enter_context(tc.tile_pool(name="tp_ps", bufs=2,
                                                  space="PSUM"))
            fps = ctx.enter_context(tc.tile_pool(name="f_ps", bufs=3,
                                                 space="PSUM"))

            hts = []

            def agg_block(b):
                gi, did, sid = ids[b]
                xo = xop.tile([P, HIDDEN], dt.bfloat16, name="xo")
                nc.sync.dma_start(xo[:], x_own[b])
                nt = max(1, min(T, -(-sum(regs[b]) // P)))
                ohs = []
                for hh in range(2):
                    oh = ohp.tile([P, T // 2, P], dt.bfloat16, name="onehot")
                    nc.vector.tensor_tensor(
                        out=oh[:],
                        in0=did[:, hh * (T // 2):(hh + 1) * (T // 2), :]
                            .to_broadcast([P, T // 2, P]),
                        in1=colidx_sb[:].to_broadcast([P, T // 2, P]),
                        op=mybir.AluOpType.is_equal)
                    ohs.append(oh)
                ps = aggps.tile([P, HIDDEN], dt.float32, space="PSUM")
                if USE_DMA_GATHER:
                    gs = []
                    for qb in range(NGB):
                        if regs[b][qb] == 0:
                            gs.append(None)
                            continue
                        g = gp.tile([P, TQ, HIDDEN], dt.bfloat16, name="gather")
                        nc.gpsimd.dma_gather(
                            g[:], x_rows[:], gi[qb], NIDX, regs[b][qb], HIDDEN,
                            single_packet=False,
                            queue_num=(NGB * b + qb) % NSWQ)
                        gs.append(g)
                    for t in range(nt):
                        nc.tensor.matmul(ps[:],
                                         lhsT=ohs[t // (T // 2)][:, t % (T // 2), :],
                                         rhs=gs[t // TQ][:, t % TQ, :],
                                         start=(t == 0), stop=(t == nt - 1))
                else:
                    for t in range(T):
                        g = gp.tile([P, HIDDEN], dt.bfloat16, name="gather")
                        nc.gpsimd.indirect_dma_start(
                            out=g[:], out_offset=None, in_=x_rows[:],
                            in_offset=bass.IndirectOffsetOnAxis(
                                ap=sid[:, t:t + 1], axis=0))
                        nc.tensor.matmul(ps[:],
                                         lhsT=ohs[t // (T // 2)][:, t % (T // 2), :],
                                         rhs=g[:],
                                         start=(t == 0), stop=(t == T - 1))
                hsb = hsbp.tile([P, HIDDEN], dt.float32, name="hsb")
                nc.vector.tensor_tensor(out=hsb[:], in0=ps[:], in1=xo[:],
                                        op=mybir.AluOpType.add)
                ht = hp.tile([P, FT, P], dt.bfloat16, name="ht")
                for j in range(FT):
                    tp = tpps.tile([P, P], dt.float32, space="PSUM")
                    nc.tensor.transpose(out=tp[:], in_=hsb[:, j * P:(j + 1) * P],
                                        identity=ident[:])
                    nc.scalar.copy(ht[:, j, :], tp[:])
                hts.append(ht)

            def fused_block(b):
                ht = hts[b]
                for v in range(NVT):
                    ps = fps.tile([P, 512], dt.float32, space="PSUM")
                    nc.tensor.matmul(
                        ps[:], lhsT=ones_sb[:],
                        rhs=wf_sb[0:1, MT_W - 1, v * 512:(v + 1) * 512],
                        start=True, stop=False)
                    for k in range(FT):
                        nc.tensor.matmul(
                            ps[:], lhsT=ht[:, k, :],
                            rhs=wf_sb[:, k, v * 512:(v + 1) * 512],
                            start=False, stop=(k == FT - 1))
                    ob = outp.tile([P, 512], dt.bfloat16, name="ob")
                    nc.scalar.copy(ob[:], ps[:])
                    nc.sync.dma_start(out_pvn[:, b, v * 512:(v + 1) * 512],
                                      ob[:])

            def emit_collective():
                nc.gpsimd.collective_compute(
                    "AllGather", mybir.AluOpType.bypass,
                    replica_groups=[list(range(NCORES))],
                    ins=[wf_slice[:].opt()],
                    outs=[wf_all[:].opt()])

            def emit_wf_load():
                for c in range(NCORES):
                    nc.sync.dma_start(wf_sb[:, :, c * VSL:(c + 1) * VSL],
                                      wf_all[c])

            for b in range(NBLK):
                if b >= DELAY:
                    fused_block(b - DELAY)
                agg_block(b)
                if b == CC_AFTER_BLOCK:
                    emit_collective()
                if b == DELAY - 1:
                    emit_wf_load()
            for b in range(NBLK - DELAY, NBLK):
                fused_block(b)

    nc.compile()
    _BUILT[key] = nc
    return nc


def _make_runner(T=T_TILES, regs=None):
    """Build (once) a cached sharded-jit callable over the 8 cores.

    Returns dict with: fn(ins_dev, outs_prev) -> outs, names, avals, mesh,
    sharding.  Outputs are donated back in as the next call's (fully
    overwritten) output buffers, so steady-state calls move no host data.
    """
    rkey = ("runner", T, None if regs is None else tuple(map(tuple, regs)))
    if rkey in _BUILT:
        return _BUILT[rkey]
    import jax
    from jax.experimental.shard_map import shard_map
    from jax.sharding import Mesh, NamedSharding, PartitionSpec
    from concourse import bass2jax, mybir

    nc = _build(T, regs)
    bass2jax.install_neuronx_cc_hook()

    pid_name = (nc.partition_id_tensor.name
                if nc.partition_id_tensor is not None else None)
    in_names, out_names, out_avals = [], [], []
    for alloc in nc.m.functions[0].allocations:
        if not isinstance(alloc, mybir.MemoryLocationSet):
            continue
        name = alloc.memorylocations[0].name
        if alloc.kind == "ExternalInput":
            if name != pid_name:
                in_names.append(name)
        elif alloc.kind == "ExternalOutput":
            out_names.append(name)
            out_avals.append(jax.core.ShapedArray(
                tuple(alloc.tensor_shape), mybir.dt.np(alloc.dtype)))
    n_params = len(in_names)
    all_names = in_names + out_names
    if pid_name is not None:
        all_names = all_names + [pid_name]
    donate = tuple(range(n_params, n_params + len(out_names)))

    def _body(*args):
        operands = list(args)
        if pid_name is not None:
            operands.append(bass2jax.partition_id_tensor())
        outs = bass2jax._bass_exec_p.bind(
            *operands,
            out_avals=tuple(out_avals),
            in_names=tuple(all_names),
            out_names=tuple(out_names),
            lowering_input_output_aliases=(),
            sim_require_finite=True,
            sim_require_nnan=True,
            nc=nc,
        )
        return tuple(outs)

    devices = jax.devices()[:NCORES]
    mesh = Mesh(np.asarray(devices), ("core",))
    spec = PartitionSpec("core")
    in_specs = (spec,) * (n_params + len(out_names))
    out_specs = (spec,) * len(out_names)
    fn = jax.jit(
        shard_map(_body, mesh=mesh, in_specs=in_specs, out_specs=out_specs,
                  check_rep=False),
        donate_argnums=donate, keep_unused=True,
    )
    sharding = NamedSharding(mesh, spec)
    runner = dict(fn=fn, in_names=in_names, out_names=out_names,
                  out_avals=out_avals, sharding=sharding, mesh=mesh)
    _BUILT[rkey] = runner
    return runner


def _prep_device_inputs(in_maps, T=T_TILES, regs=None):
    """device_put the concatenated per-core inputs; returns (ins_dev, zeros)."""
    import jax
    r = _make_runner(T, regs)
    concat = [np.concatenate([m[name] for m in in_maps], axis=0)
              for name in r["in_names"]]
    ins_dev = [jax.device_put(a, r["sharding"]) for a in concat]
    zeros = [
        jax.jit(lambda a=av: jax.numpy.zeros(
            (NCORES * a.shape[0], *a.shape[1:]), a.dtype),
            out_shardings=r["sharding"])()
        for av in r["out_avals"]
    ]
    jax.block_until_ready(ins_dev + zeros)
    return ins_dev, zeros


def _run_once(ins_dev, out_bufs, T=T_TILES, regs=None):
    import jax
    r = _make_runner(T, regs)
    outs = r["fn"](*ins_dev, *out_bufs)
    jax.block_until_ready(outs)
    return outs


def host_pack(x, edge_index, W1, b1, W2, b2):
    """Host-side packing: returns (in_maps, T)."""
    x = np.asarray(x, dtype=np.float32)
    edge_index = np.asarray(edge_index)
    W1 = np.asarray(W1, dtype=np.float32)
    b1 = np.asarray(b1, dtype=np.float32)
    W2 = np.asarray(W2, dtype=np.float32)
    b2 = np.asarray(b2, dtype=np.float32)

    src = edge_index[0].astype(np.int64)
    dst = edge_index[1].astype(np.int64)

    # --- host packing (index preprocessing + layout/dtype shuffles) ---
    x_rows = np.zeros((KSRC, HIDDEN), dtype=BF16)
    x_rows[:N_NODES] = x

    # Edge list bucketed by (core, dst-block of 128), padded to T*128 per
    # bucket.  The GIN self term (1+eps)*x_i, eps=0, is added directly from
    # x_own at PSUM evict instead of via self-loop edges.
    allsrc = src
    alldst = dst
    core = alldst // ND
    local = alldst % ND
    blk = local // P
    within = (local % P).astype(np.int32)
    bucket = core * NBLK + blk
    order = np.argsort(bucket, kind="stable")
    bs = bucket[order]
    counts = np.bincount(bucket, minlength=NCORES * NBLK)
    T = T_TILES
    maxc = int(counts.max())
    if maxc > T * P:
        T = 12 * (-(-maxc // (12 * P)))  # fallback: recompile, T /12 (6 gathers, 2 oh halves)
    cap = T * P
    starts = np.zeros(NCORES * NBLK, dtype=np.int64)
    np.cumsum(counts[:-1], out=starts[1:])
    pos = np.arange(bs.size, dtype=np.int64) - starts[bs]
    src_pad = np.full((NCORES * NBLK, cap), ZERO_ROW, dtype=np.int32)
    # pad slots get a no-match dst (is_equal -> all-zero one-hot row)
    dst_pad = np.full((NCORES * NBLK, cap), 200.0, dtype=np.float32)
    src_pad[bs, pos] = allsrc[order].astype(np.int32)
    dst_pad[bs, pos] = within[order]

    # Baked per-(block, sixth) gather counts: max over cores, rounded up to
    # 16 (idx wrap granularity).  Blocks 0-1 gather fully so every gather
    # pool buffer is written once before trimmed gathers leave stale tails.
    NGB = 6
    capq = cap // NGB
    cnt2 = counts.reshape(NCORES, NBLK)
    valid = np.clip(cnt2[:, :, None] - np.arange(NGB)[None, None, :] * capq,
                    0, capq)
    regs = valid.max(axis=0)
    regs = np.minimum(-(-regs // 16) * 16, capq)
    regs[:2, :] = capq
    # slots >= baked reg get idx -1 (skipped by the gather)
    spr = src_pad.reshape(NCORES, NBLK, NGB, capq)
    mask = np.arange(capq)[None, None, :] >= regs[:, :, None]
    spr[:, mask] = -1
    src_pad = spr.reshape(NCORES * NBLK, cap)

    # gather idxs, int16, wrapped in 16 partitions: idx i of a half-block
    # bucket lives at [i%16, i//16], replicated across the 8 Q7 cores'
    # 16-partition groups.
    seq = src_pad.reshape(NCORES, NBLK, NGB, capq // 16, 16).astype(np.int16)
    seq = seq.transpose(0, 1, 2, 4, 3)          # [core, b, qb, 16, capq/16]
    gidx = np.broadcast_to(
        seq[:, :, :, None, :, :],
        (NCORES, NBLK, NGB, P // 16, 16, capq // 16))
    gidx = np.ascontiguousarray(
        gidx.reshape(NCORES, NBLK, NGB, P, capq // 16).transpose(0, 1, 3, 2, 4))

    # per-tile int32 src ids (for the indirect-DMA fallback gather path)
    src_i32 = src_pad.reshape(NCORES, NBLK, T, P).transpose(0, 1, 3, 2)
    src_i32 = np.ascontiguousarray(src_i32)

    # one-hot dst ids: [core, blk, P, T, 1] bf16 (tile t at [., ., :, t, 0])
    dst_pad = dst_pad.reshape(NCORES, NBLK, T, P).transpose(0, 1, 3, 2)
    dst_pad = np.ascontiguousarray(dst_pad.astype(BF16))[..., None]

    colidx = np.broadcast_to(np.arange(P, dtype=np.float32)[None, :], (P, P))
    colidx = np.ascontiguousarray(colidx.astype(BF16))

    # W1 with b1 replicated into 128 extra columns (bias tile of Wf^T)
    w1b = np.concatenate(
        [W1, np.broadcast_to(b1[:, None], (MIDDLE, P))], axis=1)
    w1b_kxm = _pack(w1b.astype(BF16))                       # [P, 34, 640]
    w2t = W2.T.astype(BF16)                                 # [4352, 8192]

    x_own = np.zeros((NCORES, NBLK * P, HIDDEN), dtype=BF16)
    x_own[:, :ND] = x.reshape(NCORES, ND, HIDDEN).astype(BF16)
    x_own = x_own.reshape(NCORES, NBLK, P, HIDDEN)

    in_maps = []
    for c in range(NCORES):
        vsl = slice(c * VSL, (c + 1) * VSL)
        in_maps.append({
            "x_rows": x_rows,
            "x_own": np.ascontiguousarray(x_own[c]),
            "gidx": np.ascontiguousarray(gidx[c]),
            "src_ids": src_i32[c],
            "dst_ids": dst_pad[c],
            "colidx": colidx,
            "w1b_kxm": w1b_kxm,
            "w2t_kxn": _pack(np.ascontiguousarray(w2t[:, vsl])),
            "b2rep": np.ascontiguousarray(np.broadcast_to(
                b2[vsl].astype(BF16)[None, :], (P, VSL))),
        })
    return in_maps, T, [tuple(int(v) for v in row) for row in regs]


def kernel(x, edge_index, W1, b1, W2, b2):
    global LAST_RESULTS

    in_maps, T, regs = host_pack(x, edge_index, W1, b1, W2, b2)

    ins_dev, zeros = _prep_device_inputs(in_maps, T, regs)
    outs = _run_once(ins_dev, zeros, T, regs)
    LAST_RESULTS = dict(ins_dev=ins_dev, outs=outs, T=T, regs=regs)

    r = _make_runner(T, regs)
    out_global = np.asarray(outs[r["out_names"].index("out_pvn")])
    out_global = out_global.reshape(NCORES, P, NBLK, VOCAB)

    out = np.empty((N_NODES, VOCAB), dtype=np.float32)
    for c in range(NCORES):
        o = _unpack(out_global[c])                          # [2560, 8192]
        out[c * ND:(c + 1) * ND] = o[:ND].astype(np.float32)
    return out


def bench(iters=5):
    """Steady-state per-call wall time (s); requires kernel() to have run."""
    import time
    import jax
    st = LAST_RESULTS
    assert st is not None, "run kernel() first"
    outs = st["outs"]
    times = []
    for _ in range(iters):
        t0 = time.perf_counter()
        outs = _run_once(st["ins_dev"], outs, st["T"], st["regs"])
        times.append(time.perf_counter() - t0)
    st["outs"] = outs
    return times


def bench_pipelined(iters=8):
    """Dispatch `iters` chained calls without blocking, block once.

    Successive calls are serialized on-device by the donated-output data
    dependence, while host dispatch overlaps — the per-iter slope is the
    device execution time.
    """
    import time
    import jax
    st = LAST_RESULTS
    assert st is not None, "run kernel() first"
    r = _make_runner(st["T"], st["regs"])
    outs = st["outs"]
    # warm: one blocked call so everything is resident
    outs = _run_once(st["ins_dev"], outs, st["T"], st["regs"])
    t0 = time.perf_counter()
    outs = r["fn"](*st["ins_dev"], *outs)
    jax.block_until_ready(outs)
    t1 = time.perf_counter() - t0
    t0 = time.perf_counter()
    for _ in range(iters):
        outs = r["fn"](*st["ins_dev"], *outs)
    jax.block_until_ready(outs)
    tN = time.perf_counter() - t0
    st["outs"] = outs
    per_iter = (tN - t1) / (iters - 1)
    return dict(t1=t1, tN=tN, iters=iters, per_iter=per_iter)
